# revision 17
# baseline (speedup 1.0000x reference)
"""Causal multi-head attention block (GPT-style) for Trainium2, 8 NeuronCores.

Problem: x[4,2048,768] -> qkv = x@W_attn+b_attn -> 12-head causal attention
         -> y@W_proj+b_proj -> out[4,2048,768]   (fp32 I/O)

Sharding: 4 batches x 2 head-groups (6 heads each). c_attn column-sharded,
c_proj row-sharded over head groups; ReduceScatter(add) over core pairs after
c_proj (even core keeps summed rows 0:1024, odd core rows 1024:2048; host
stitches). Core c = 2*b + g handles batch b, heads 6g..6g+5.

Per-core kernel:
  1. QKV^T = Wa_g^T @ x^T (bf16 matmuls): Q^T,K^T [384,2048] pair-packed
     col-tile-major; V [2048,384] stored fp8e4 in j-PAIR layout
     [128, head, 2, 64] with pair order (odd s-tile, even s-tile).
  2. Flash-style causal attention in transposed-score orientation:
     S^T[k,q] blocks via row-packed pair matmuls (K=64 head A rows 0-63 /
     head B rows 64-127) in bf16, j-tiles grouped in PAIRS with block
     order (odd j, even j) so diagonal pairs have contiguous valid
     regions; exp on ScalarE (PSUM->SBUF fp8e4, scale=1/8); stair
     masking via 128-col mask multiplies (pad cols are startup-zeroed
     pool buffers); y_u^T and the softmax normalizer n via fp8
     DoubleRow matmuls (two j-tiles contracted per instruction, 0.5
     cycles/row): lhsT = V-pair [128,2,64] / replicated-ones [128,2,64],
     accumulating into av[64, 0:512]=y_u, av[64, 512:1024]=n-broadcast.
  3. Normalize elementwise: ytp = y_u * reciprocal(n) (n already
     partition-replicated by the ones-matmul, so no broadcast needed).
  4. proj: out_partial[s,768] = sum_pairs yT_pair^T @ Wp_pair, + b_proj.
  5. Three chunked ReduceScatters over {2b,2b+1} ([0:1024] after i=1,
     [1024:1536] after i=2, [1536:2048] at the end), each ~15us modeled,
     results bounced rs_out -> SBUF -> out.

The walrus build here allows only one sync-wait per instruction; a post-pass
(legalize_waits) hoists extra waits onto single-wait NOPs.
"""
import numpy as np
import ml_dtypes

import concourse.bass as bass
import concourse.tile as tile
from concourse import mybir
from concourse.bass_utils import run_bass_kernel_spmd
from concourse import mybir as mb

BF16 = mybir.dt.bfloat16
F32 = mybir.dt.float32
FP8 = mybir.dt.float8e4
DR = mybir.MatmulPerfMode.DoubleRow

B, S, D = 4, 2048, 768
H, HD = 12, 64
G = 2                 # head groups
HL = H // G           # heads per core = 6
DL = HL * HD          # local head dims = 384
NP = HL // 2          # head pairs per core = 3
P = 128
QT = 512              # q tile
NI = S // QT          # 4 q tiles
NDT = D // P          # 6 D tiles
N_CORES = 8
REPEAT = 1


def _legalize_waits(nc):
    n_split = 0
    for f in nc.m.functions:
        for bb in f.blocks:
            insts = list(bb.instructions)
            out = []
            changed = False
            for inst in insts:
                si = inst.sync_info
                if si is not None:
                    waits = list(si.on_wait)
                    if len(waits) > 1:
                        for w in waits[:-1]:
                            nop = mb.InstNoOp(name=f"I-wsplit-{nc.next_id()}", ins=[], outs=[])
                            nop.engine = inst.engine
                            nop.sync_info = mb.SyncInfo(on_wait=[w], on_update=[])
                            out.append(nop)
                            n_split += 1
                        inst.sync_info = mb.SyncInfo(on_wait=[waits[-1]], on_update=list(si.on_update))
                        changed = True
                out.append(inst)
            if changed:
                bb.instructions = out
    return n_split


def _build():
    nc = bass.Bass("TRN2", target_bir_lowering=False, debug=False, num_devices=N_CORES)

    xT = nc.dram_tensor("xT", [D, S], BF16, kind="ExternalInput").ap()
    wa = nc.dram_tensor("wa", [D, 3 * DL], BF16, kind="ExternalInput").ap()
    ba = nc.dram_tensor("ba", [P, 9], F32, kind="ExternalInput").ap()
    bv = nc.dram_tensor("bv", [P, DL], F32, kind="ExternalInput").ap()
    wp = nc.dram_tensor("wp", [DL, D], BF16, kind="ExternalInput").ap()
    bp = nc.dram_tensor("bp", [P, D], F32, kind="ExternalInput").ap()
    msk = nc.dram_tensor("msk", [P, P], FP8, kind="ExternalInput").ap()
    out = nc.dram_tensor("out", [S // 2, D], F32, kind="ExternalOutput").ap()
    # ReduceScatter chunks: separate in-tensors so proj writes emitted after
    # an RS carry no false WAR dep; internal outs (collectives can't write IO)
    chunk_rows = [(0, 1024), (1024, 1536), (1536, 2048)]
    ar_ins = [
        nc.dram_tensor(f"ar_in{c}", [r1 - r0, D], F32).ap()
        for c, (r0, r1) in enumerate(chunk_rows)
    ]
    rs_outs = [
        nc.dram_tensor(f"rs_out{c}", [(r1 - r0) // 2, D], F32).ap()
        for c, (r0, r1) in enumerate(chunk_rows)
    ]

    with tile.TileContext(nc) as tc:
        with (
            tc.tile_pool(name="wgt", bufs=1) as wpool,
            tc.tile_pool(name="qkv", bufs=1) as qkvpool,
            tc.tile_pool(name="pt", bufs=4) as ptpool,
            tc.tile_pool(name="ptd", bufs=2) as ptdpool,
            tc.tile_pool(name="yt", bufs=2) as ytpool,
            tc.tile_pool(name="nrm", bufs=3) as nrmpool,
            tc.tile_pool(name="ob", bufs=3) as obpool,
            tc.tile_pool(name="scp", bufs=2, space="PSUM") as scpool,
            tc.tile_pool(name="avp", bufs=1, space="PSUM") as avpool,
        ):
            # ---- phase 0: load weights/constants ----
            # interleave wa/x tile loads so the first QKV matmuls (which
            # consume wak[t]/xk[t] in t order) start as early as possible;
            # DMA transfers serialize on the shared DMA-engine device.
            wak, xk = [], []
            for t in range(NDT):
                wt_sb = wpool.tile([P, 3 * DL], BF16, tag=f"wak{t}")
                nc.sync.dma_start(wt_sb[:], wa[bass.ts(t, P), :])
                wak.append(wt_sb)
                xt_sb = wpool.tile([P, S], BF16, tag=f"xk{t}")
                nc.sync.dma_start(xt_sb[:, 0 : S // 2], xT[bass.ts(t, P), 0 : S // 2])
                xk.append(xt_sb)
            for t in range(NDT):
                nc.sync.dma_start(xk[t][:, S // 2 :], xT[bass.ts(t, P), S // 2 :])
            wpp = []
            for p in range(NP):
                wp_sb = wpool.tile([P, D], BF16, tag=f"wpp{p}")
                nc.sync.dma_start(wp_sb[:], wp[bass.ts(p, P), :])
                wpp.append(wp_sb)
            ba_sb = wpool.tile([P, 9], F32, tag="ba")
            nc.sync.dma_start(ba_sb[:], ba[:])
            bv_sb = wpool.tile([P, DL], F32, tag="bv")
            nc.sync.dma_start(bv_sb[:], bv[:])
            bp_sb = wpool.tile([P, D], F32, tag="bp")
            nc.sync.dma_start(bp_sb[:], bp[:])
            msk_sb = wpool.tile([P, P], FP8, tag="msk")
            nc.sync.dma_start(msk_sb[:], msk[:])
            mskb_sb = wpool.tile([P, P], BF16, tag="mskb")
            nc.vector.tensor_copy(mskb_sb[:], msk_sb[:])
            ones2_sb = wpool.tile([P, P], FP8, tag="ones2")
            nc.vector.memset(ones2_sb[:], 1.0)
            ones2v = ones2_sb[:].rearrange("p (t m) -> p t m", t=2)
            onesb_sb = wpool.tile([P, HD], BF16, tag="onesb")
            nc.vector.memset(onesb_sb[:], 1.0)
            # pre-zero the pad region [0:128] of every diag-pt pool buffer;
            # steady state keeps it zero (only read afterwards)
            for _b in range(2):
                t1 = ptdpool.tile([P, 1024], FP8, tag="ptd1")
                nc.vector.memset(t1[:, 0:P], 0.0)
                t2 = ptdpool.tile([P, 512], FP8, tag="ptd2")
                nc.vector.memset(t2[:, 0:P], 0.0)
            # prewarm ScalarE's exp table set during the QKV phase so the
            # first attention exp doesn't pay the ACT_TABLE_LOAD
            warm_sb = wpool.tile([1, 2], F32, tag="warm")
            nc.vector.memset(warm_sb[:], 0.0)
            nc.scalar.activation(warm_sb[:, 1:2], warm_sb[:, 0:1],
                                 mybir.ActivationFunctionType.Exp)

            # ---- phase 1: Q^T, K^T  (col-tile m: 0-2 = Q pairs, 3-5 = K pairs)
            qt_t, kt_t = [None] * NP, [None] * NP

            def emit_qk(m, n2):
                if n2 == 0:
                    dst = qkvpool.tile([P, S], BF16, tag=f"qkvT{m}")
                    if m < NP:
                        qt_t[m] = dst
                    else:
                        kt_t[m - NP] = dst
                else:
                    dst = qt_t[m] if m < NP else kt_t[m - NP]
                ps = scpool.tile([P, 1024], F32, tag="sc")
                for half in range(2):
                    n = 2 * n2 + half
                    for t in range(NDT):
                        nc.tensor.matmul(
                            ps[:, bass.ts(half, QT)],
                            lhsT=wak[t][:, bass.ts(m, P)],
                            rhs=xk[t][:, bass.ts(n, QT)],
                            start=(t == 0),
                            stop=(t == NDT - 1),
                        )
                for half in range(2):
                    n = 2 * n2 + half
                    nc.vector.tensor_scalar_add(
                        dst[:, bass.ts(n, QT)],
                        ps[:, bass.ts(half, QT)],
                        ba_sb[:, m : m + 1],
                    )

            # V j-pair tiles (fp8): v_t[u] holds s-tiles (2u+1, 2u) in free
            # slots (t=0 -> odd, t=1 -> even) to match the score-pair order.
            # u 0/1 also get bf16 copies: q-tile i=0 (few attended keys, so
            # quantization noise doesn't average out) runs a bf16 AV path.
            v_t = [None] * (S // P // 2)
            v8_t = [None, None]

            def emit_v(s):
                ps = scpool.tile([P, 1024], F32, tag="sc")
                for t in range(NDT):
                    nc.tensor.matmul(
                        ps[:, 0:DL],
                        lhsT=xk[t][:, bass.ts(s, P)],
                        rhs=wak[t][:, 2 * DL : 3 * DL],
                        start=(t == 0),
                        stop=(t == NDT - 1),
                    )
                u, odd = divmod(s, 2)
                if v_t[u] is None:
                    v_t[u] = qkvpool.tile([P, HL * 2 * HD], FP8, tag=f"v{u}", name=f"v{u}")
                vt4 = v_t[u][:].rearrange("p (h t x) -> p h t x", h=HL, t=2)
                nc.vector.tensor_add(
                    vt4[:, :, 0 if odd else 1, :],
                    ps[:, 0:DL].rearrange("p (h x) -> p h x", h=HL),
                    bv_sb[:].rearrange("p (h x) -> p h x", h=HL),
                )
                if u < 2:
                    if v8_t[u] is None:
                        v8_t[u] = qkvpool.tile([P, HL * 2 * HD], BF16, tag=f"v8{u}", name=f"v8{u}")
                    v8t4 = v8_t[u][:].rearrange("p (h t x) -> p h t x", h=HL, t=2)
                    nc.vector.tensor_add(
                        v8t4[:, :, 0 if odd else 1, :],
                        ps[:, 0:DL].rearrange("p (h x) -> p h x", h=HL),
                        bv_sb[:].rearrange("p (h x) -> p h x", h=HL),
                    )

            yt_t = [None] * NP

            def emit_attn(i, p):
                hA, hB = 2 * p, 2 * p + 1
                avA = avpool.tile([HD, 1024], F32, tag="avA")
                avB = avpool.tile([HD, 1024], F32, tag="avB")
                n_pairs = 2 * i + 2
                scs = {}

                # score blocks per pair u: (j, col_off, q0, n); block order is
                # (odd j, even j) to match the V-pair layout and to make the
                # diagonal pairs' valid region contiguous
                def blocks_of(u):
                    if u < 2 * i:
                        return [(2 * u + 1, 0, i * QT, QT), (2 * u, QT, i * QT, QT)]
                    if u == 2 * i:
                        return [(4 * i + 1, P, i * QT + P, 384), (4 * i, QT, i * QT, QT)]
                    return [(4 * i + 3, P, i * QT + 384, P), (4 * i + 2, 256, i * QT + 256, 256)]

                def emit_scores(u):
                    scA = scpool.tile([P, 1024], F32, tag="sc")
                    scB = scpool.tile([P, 1024], F32, tag="sc")
                    for j, off, q0, n in blocks_of(u):
                        for sc, lo in ((scA, 0), (scB, HD)):
                            nc.tensor.matmul(
                                sc[:, off : off + n],
                                lhsT=kt_t[p][lo : lo + HD, bass.ts(j, P)],
                                rhs=qt_t[p][lo : lo + HD, q0 : q0 + n],
                                start=True, stop=True,
                            )
                    scs[u] = (scA, scB)

                def process(u):
                    scA, scB = scs.pop(u)
                    bf = i == 0  # bf16 AV path for the first q-tile
                    PDT = BF16 if bf else FP8
                    mtile = mskb_sb if bf else msk_sb
                    if u < 2 * i:
                        e0, e1 = 0, 1024
                        ptA = ptpool.tile([P, 1024], PDT, tag="pt")
                        ptB = ptpool.tile([P, 1024], PDT, tag="pt")
                        masks = []
                        qoff, qn = 0, QT
                    elif u == 2 * i:
                        e0, e1 = P, 1024
                        ptA = ptdpool.tile([P, 1024], PDT, tag="ptd1b" if bf else "ptd1")
                        ptB = ptdpool.tile([P, 1024], PDT, tag="ptd1b" if bf else "ptd1")
                        masks = [(P, 256), (QT, QT + P)]
                        qoff, qn = 0, QT
                    else:
                        e0, e1 = P, 512
                        ptA = ptdpool.tile([P, 512], PDT, tag="ptd2b" if bf else "ptd2")
                        ptB = ptdpool.tile([P, 512], PDT, tag="ptd2b" if bf else "ptd2")
                        masks = [(P, 256), (256, 384)]
                        qoff, qn = 256, 256
                    for sc, pt in ((scA, ptA), (scB, ptB)):
                        nc.scalar.activation(pt[:, e0:e1], sc[:, e0:e1],
                                             mybir.ActivationFunctionType.Exp, scale=0.125)
                        for m0, m1 in masks:
                            nc.vector.tensor_mul(pt[:, m0:m1], pt[:, m0:m1], mtile[:])
                    first, last = (u == 0), (u == n_pairs - 1)
                    if bf:
                        # plain bf16 matmuls over the exact valid block ranges
                        # (block order: j0-full first so `start` covers the
                        # whole region)
                        v8t4 = v8_t[u][:].rearrange("p (h t x) -> p h t x", h=HL, t=2)
                        if u == 0:
                            blks = [(1, QT, 0, QT), (0, P, P, 384)]
                        else:
                            blks = [(1, 256, 256, 256), (0, P, 384, P)]
                        for pt, av, h in ((ptA, avA, hA), (ptB, avB, hB)):
                            for bi, (t, off, ql, n) in enumerate(blks):
                                st = first and bi == 0
                                sp = last and bi == len(blks) - 1
                                nc.tensor.matmul(
                                    av[:, ql : ql + n],
                                    lhsT=v8t4[:, h, t, :], rhs=pt[:, off : off + n],
                                    start=st, stop=sp,
                                )
                                nc.tensor.matmul(
                                    av[:, QT + ql : QT + ql + n],
                                    lhsT=onesb_sb[:], rhs=pt[:, off : off + n],
                                    start=st, stop=sp,
                                )
                    else:
                        for pt, av, h in ((ptA, avA, hA), (ptB, avB, hB)):
                            rhs = pt[:, 0 : 2 * qn].rearrange("p (t n) -> p t n", t=2)
                            vt4 = v_t[u][:].rearrange("p (h t x) -> p h t x", h=HL, t=2)
                            nc.tensor.matmul(
                                av[:, qoff : qoff + qn], lhsT=vt4[:, h, :, :], rhs=rhs,
                                start=first, stop=last, perf_mode=DR,
                            )
                            nc.tensor.matmul(
                                av[:, QT + qoff : QT + qoff + qn], lhsT=ones2v, rhs=rhs,
                                start=first, stop=last, perf_mode=DR,
                            )

                emit_scores(0)
                for u in range(n_pairs):
                    if u + 1 < n_pairs:
                        emit_scores(u + 1)
                    process(u)

                # normalize: n is partition-replicated in av[:, 512:1024], so
                # ytp = y_u * recip(n) elementwise; head B bounced via DMA to
                # reach partitions 64:128
                ytp = ytpool.tile([P, QT], BF16, tag=f"yt{p}")
                rbA = nrmpool.tile([HD, QT], F32, tag="rbA")
                nc.vector.reciprocal(rbA[:], avA[:, QT:])
                nc.vector.tensor_mul(ytp[0:HD, :], avA[:, 0:QT], rbA[:])
                rbB = nrmpool.tile([HD, QT], F32, tag="rbB")
                nc.vector.reciprocal(rbB[:], avB[:, QT:])
                tmpB = nrmpool.tile([HD, QT], BF16, tag="tmpB")
                nc.vector.tensor_mul(tmpB[:], avB[:, 0:QT], rbB[:])
                nc.sync.dma_start(ytp[HD:P, :], tmpB[:])
                yt_t[p] = ytp

            def emit_proj(i):
                for ss in range(QT // P):
                    ps = scpool.tile([P, 1024], F32, tag="sc")
                    for p in range(NP):
                        nc.tensor.matmul(
                            ps[:, 0:512],
                            lhsT=yt_t[p][:, bass.ts(ss, P)],
                            rhs=wpp[p][:, 0:512],
                            start=(p == 0), stop=(p == NP - 1),
                        )
                        nc.tensor.matmul(
                            ps[:, 512:768],
                            lhsT=yt_t[p][:, bass.ts(ss, P)],
                            rhs=wpp[p][:, 512:768],
                            start=(p == 0), stop=(p == NP - 1),
                        )
                    ob = obpool.tile([P, D], F32, tag="ob")
                    nc.vector.tensor_add(ob[:], ps[:, 0:D], bp_sb[:])
                    row0 = i * QT + ss * P
                    for c, (r0, r1) in enumerate(chunk_rows):
                        if r0 <= row0 < r1:
                            nc.sync.dma_start(ar_ins[c][row0 - r0 : row0 - r0 + P, :], ob[:])

            def emit_rs(c):
                # ReduceScatter(add) over the core pair: even core receives the
                # summed first half of the chunk, odd core the second half;
                # result bounced through SBUF into `out` (host stitches)
                nc.gpsimd.collective_compute(
                    "ReduceScatter",
                    mybir.AluOpType.add,
                    replica_groups=[[0, 1], [2, 3], [4, 5], [6, 7]],
                    ins=[ar_ins[c][:, :].opt()],
                    outs=[rs_outs[c][:, :].opt()],
                )
                base = [0, 512, 768][c]
                nrows = (chunk_rows[c][1] - chunk_rows[c][0]) // 2
                for r in range(0, nrows, P):
                    oc = obpool.tile([P, D], F32, tag="oc")
                    nc.sync.dma_start(oc[:], rs_outs[c][r : r + P, :])
                    nc.sync.dma_start(out[base + r : base + r + P, :], oc[:])

            # ---- main interleaved schedule ----
            # attention for q-tile i only needs Q/K cols [0:(i+1)*512] and V
            # s-tiles [0:4(i+1)], so QKV emission interleaves with attention:
            # the ScalarE exp stream starts ~40us earlier and the QKV matmuls
            # fill PE stalls in the Act-paced attention phase
            for _rep in range(REPEAT):
                emit_qk(0, 0)
                emit_qk(3, 0)
                for s in range(4):
                    emit_v(s)
                emit_attn(0, 0)
                emit_qk(1, 0)
                emit_qk(4, 0)
                emit_attn(0, 1)
                emit_qk(2, 0)
                emit_qk(5, 0)
                emit_attn(0, 2)
                for s in range(4, 8):
                    emit_v(s)
                emit_proj(0)
                emit_attn(1, 0)
                emit_qk(0, 1)
                emit_qk(3, 1)
                emit_attn(1, 1)
                emit_qk(1, 1)
                emit_qk(4, 1)
                emit_attn(1, 2)
                emit_qk(2, 1)
                emit_qk(5, 1)
                emit_proj(1)
                emit_rs(0)
                for s in range(8, 12):
                    emit_v(s)
                for p in range(NP):
                    emit_attn(2, p)
                emit_proj(2)
                emit_rs(1)
                for s in range(12, 16):
                    emit_v(s)
                for p in range(NP):
                    emit_attn(3, p)
                emit_proj(3)
                emit_rs(2)

    _legalize_waits(nc)
    return nc


_NC_CACHE = {}


def _get_nc():
    if "nc" not in _NC_CACHE:
        _NC_CACHE["nc"] = _build()
    return _NC_CACHE["nc"]


def _prep_inputs(x, W_attn, b_attn, W_proj, b_proj):
    bf = ml_dtypes.bfloat16
    fp8 = ml_dtypes.float8_e4m3
    x = np.asarray(x, np.float32)
    W_attn = np.asarray(W_attn, np.float32)
    b_attn = np.asarray(b_attn, np.float32)
    W_proj = np.asarray(W_proj, np.float32)
    b_proj = np.asarray(b_proj, np.float32)

    # stair mask for the 128-col diagonal of each diag j-block
    mask = (np.arange(P)[None, :] >= np.arange(P)[:, None]).astype(fp8)

    in_maps = []
    for c in range(N_CORES):
        b, g = divmod(c, 2)
        cols = slice(DL * g, DL * g + DL)
        xT = np.ascontiguousarray(x[b].T).astype(bf)
        wa = np.concatenate(
            [W_attn[:, 0:D][:, cols], W_attn[:, D : 2 * D][:, cols], W_attn[:, 2 * D :][:, cols]],
            axis=1,
        ).astype(bf)
        ba_sl = np.concatenate(
            [b_attn[0:D][cols], b_attn[D : 2 * D][cols], b_attn[2 * D :][cols]]
        ).astype(np.float32)
        ba2 = np.ascontiguousarray(ba_sl[: 2 * DL].reshape(6, P).T)
        ba9 = np.zeros((P, 9), np.float32)
        ba9[:, :6] = ba2
        bv_b = np.ascontiguousarray(np.broadcast_to(ba_sl[2 * DL :], (P, DL))).astype(np.float32)
        wp_c = np.ascontiguousarray(W_proj[cols, :]).astype(bf)
        bp_full = b_proj if g == 0 else np.zeros_like(b_proj)
        bp_b = np.ascontiguousarray(np.broadcast_to(bp_full, (P, D))).astype(np.float32)
        in_maps.append(
            {
                "xT": xT,
                "wa": wa,
                "ba": ba9,
                "bv": bv_b,
                "wp": wp_c,
                "bp": bp_b,
                "msk": mask,
            }
        )
    return in_maps


def kernel(x, W_attn, b_attn, W_proj, b_proj):
    in_maps = _prep_inputs(x, W_attn, b_attn, W_proj, b_proj)
    nc = _get_nc()
    res = run_bass_kernel_spmd(nc, in_maps, list(range(N_CORES)))
    # stitch: chunk c (global rows r0:r1) -> even core's out rows
    # [base:base+h] = summed r0:r0+h, odd core's = r0+h:r1
    outs = []
    for b in range(B):
        rows = []
        for c, (r0, r1) in enumerate([(0, 1024), (1024, 1536), (1536, 2048)]):
            h = (r1 - r0) // 2
            base = [0, 512, 768][c]
            rows.append(res.results[2 * b]["out"][base : base + h])
            rows.append(res.results[2 * b + 1]["out"][base : base + h])
        outs.append(np.concatenate(rows, axis=0))
    return np.stack(outs).astype(np.float32)


# revision 23
# speedup vs baseline: 1.0767x; 1.0767x over previous
"""Causal multi-head attention block (GPT-style) for Trainium2, 8 NeuronCores.

Problem: x[4,2048,768] -> qkv = x@W_attn+b_attn -> 12-head causal attention
         -> y@W_proj+b_proj -> out[4,2048,768]   (fp32 I/O)

Sharding: 4 batches x 2 head-groups (6 heads each). c_attn column-sharded,
c_proj row-sharded over head groups; ReduceScatter(add) over core pairs after
c_proj (even core keeps summed rows 0:1024, odd core rows 1024:2048; host
stitches). Core c = 2*b + g handles batch b, heads 6g..6g+5.

Per-core kernel:
  1. QKV^T = Wa_g^T @ x^T (bf16 matmuls): Q^T,K^T [384,2048] pair-packed
     col-tile-major; V [2048,384] stored fp8e4 in j-PAIR layout
     [128, head, 2, 64] with pair order (odd s-tile, even s-tile).
  2. Flash-style causal attention in transposed-score orientation:
     S^T[k,q] blocks via row-packed pair matmuls (K=64 head A rows 0-63 /
     head B rows 64-127) in bf16, j-tiles grouped in PAIRS with block
     order (odd j, even j) so diagonal pairs have contiguous valid
     regions; exp on ScalarE (PSUM->SBUF fp8e4, scale=1/8); stair
     masking via 128-col mask multiplies (pad cols are startup-zeroed
     pool buffers); y_u^T and the softmax normalizer n via fp8
     DoubleRow matmuls (two j-tiles contracted per instruction, 0.5
     cycles/row): lhsT = V-pair [128,2,64] / replicated-ones [128,2,64],
     accumulating into av[64, 0:512]=y_u, av[64, 512:1024]=n-broadcast.
  3. Normalize elementwise: ytp = y_u * reciprocal(n) (n already
     partition-replicated by the ones-matmul, so no broadcast needed).
  4. proj: out_partial[s,768] = sum_pairs yT_pair^T @ Wp_pair, + b_proj.
  5. Three chunked ReduceScatters over {2b,2b+1} ([0:1024] after i=1,
     [1024:1536] after i=2, [1536:2048] at the end), each ~15us modeled,
     results bounced rs_out -> SBUF -> out.

The walrus build here allows only one sync-wait per instruction; a post-pass
(legalize_waits) hoists extra waits onto single-wait NOPs.
"""
import numpy as np
import ml_dtypes

import concourse.bass as bass
import concourse.tile as tile
from concourse import mybir
from concourse.bass_utils import run_bass_kernel_spmd
from concourse import mybir as mb

BF16 = mybir.dt.bfloat16
F32 = mybir.dt.float32
FP8 = mybir.dt.float8e4
DR = mybir.MatmulPerfMode.DoubleRow

B, S, D = 4, 2048, 768
H, HD = 12, 64
G = 2                 # head groups
HL = H // G           # heads per core = 6
DL = HL * HD          # local head dims = 384
NP = HL // 2          # head pairs per core = 3
P = 128
QT = 512              # q tile
NI = S // QT          # 4 q tiles
NDT = D // P          # 6 D tiles
N_CORES = 8
REPEAT = 1


def _legalize_waits(nc):
    n_split = 0
    for f in nc.m.functions:
        for bb in f.blocks:
            insts = list(bb.instructions)
            out = []
            changed = False
            for inst in insts:
                si = inst.sync_info
                if si is not None:
                    waits = list(si.on_wait)
                    if len(waits) > 1:
                        for w in waits[:-1]:
                            nop = mb.InstNoOp(name=f"I-wsplit-{nc.next_id()}", ins=[], outs=[])
                            nop.engine = inst.engine
                            nop.sync_info = mb.SyncInfo(on_wait=[w], on_update=[])
                            out.append(nop)
                            n_split += 1
                        inst.sync_info = mb.SyncInfo(on_wait=[waits[-1]], on_update=list(si.on_update))
                        changed = True
                out.append(inst)
            if changed:
                bb.instructions = out
    return n_split


def _build():
    nc = bass.Bass("TRN2", target_bir_lowering=False, debug=False, num_devices=N_CORES)

    xT = nc.dram_tensor("xT", [D, S], BF16, kind="ExternalInput").ap()
    wa = nc.dram_tensor("wa", [D, 3 * DL], BF16, kind="ExternalInput").ap()
    ba = nc.dram_tensor("ba", [P, 9], F32, kind="ExternalInput").ap()
    bv = nc.dram_tensor("bv", [P, DL], F32, kind="ExternalInput").ap()
    wp = nc.dram_tensor("wp", [DL, D], BF16, kind="ExternalInput").ap()
    bp = nc.dram_tensor("bp", [P, D], F32, kind="ExternalInput").ap()
    msk = nc.dram_tensor("msk", [P, P], FP8, kind="ExternalInput").ap()
    out = nc.dram_tensor("out", [S // 2, D], F32, kind="ExternalOutput").ap()
    # ReduceScatter chunks: separate in-tensors so proj writes emitted after
    # an RS carry no false WAR dep; internal outs (collectives can't write IO)
    chunk_rows = [(0, 1024), (1024, 1536), (1536, 2048)]
    ar_ins = [
        nc.dram_tensor(f"ar_in{c}", [r1 - r0, D], F32).ap()
        for c, (r0, r1) in enumerate(chunk_rows)
    ]
    rs_outs = [
        nc.dram_tensor(f"rs_out{c}", [(r1 - r0) // 2, D], F32).ap()
        for c, (r0, r1) in enumerate(chunk_rows)
    ]

    with tile.TileContext(nc) as tc:
        with (
            tc.tile_pool(name="wgt", bufs=1) as wpool,
            tc.tile_pool(name="qkv", bufs=1) as qkvpool,
            tc.tile_pool(name="pt", bufs=4) as ptpool,
            tc.tile_pool(name="ptd", bufs=2) as ptdpool,
            tc.tile_pool(name="yt", bufs=2) as ytpool,
            tc.tile_pool(name="nrm", bufs=3) as nrmpool,
            tc.tile_pool(name="ob", bufs=3) as obpool,
            tc.tile_pool(name="scp", bufs=2, space="PSUM") as scpool,
            tc.tile_pool(name="avp", bufs=1, space="PSUM") as avpool,
        ):
            # ---- phase 0: load weights/constants ----
            # interleave wa/x tile loads so the first QKV matmuls (which
            # consume wak[t]/xk[t] in t order) start as early as possible;
            # DMA transfers serialize on the shared DMA-engine device.
            wak, xk = [], []
            for t in range(NDT):
                wt_sb = wpool.tile([P, 3 * DL], BF16, tag=f"wak{t}")
                nc.sync.dma_start(wt_sb[:], wa[bass.ts(t, P), :])
                wak.append(wt_sb)
                xt_sb = wpool.tile([P, S], BF16, tag=f"xk{t}")
                nc.sync.dma_start(xt_sb[:, 0 : S // 2], xT[bass.ts(t, P), 0 : S // 2])
                xk.append(xt_sb)
            for t in range(NDT):
                nc.sync.dma_start(xk[t][:, S // 2 :], xT[bass.ts(t, P), S // 2 :])
            wpp = []
            for p in range(NP):
                wp_sb = wpool.tile([P, D], BF16, tag=f"wpp{p}")
                nc.sync.dma_start(wp_sb[:], wp[bass.ts(p, P), :])
                wpp.append(wp_sb)
            ba_sb = wpool.tile([P, 9], F32, tag="ba")
            nc.sync.dma_start(ba_sb[:], ba[:])
            bv_sb = wpool.tile([P, DL], F32, tag="bv")
            nc.sync.dma_start(bv_sb[:], bv[:])
            bp_sb = wpool.tile([P, D], F32, tag="bp")
            nc.sync.dma_start(bp_sb[:], bp[:])
            msk_sb = wpool.tile([P, P], FP8, tag="msk")
            nc.sync.dma_start(msk_sb[:], msk[:])
            mskb_sb = wpool.tile([P, P], BF16, tag="mskb")
            nc.vector.tensor_copy(mskb_sb[:], msk_sb[:])
            ones2_sb = wpool.tile([P, P], FP8, tag="ones2")
            nc.vector.memset(ones2_sb[:], 1.0)
            ones2v = ones2_sb[:].rearrange("p (t m) -> p t m", t=2)
            onesb_sb = wpool.tile([P, HD], BF16, tag="onesb")
            nc.vector.memset(onesb_sb[:], 1.0)
            # pre-zero the pad region [0:128] of every diag-pt pool buffer;
            # steady state keeps it zero (only read afterwards)
            for _b in range(2):
                t1 = ptdpool.tile([P, 1024], FP8, tag="ptd1")
                nc.vector.memset(t1[:, 0:P], 0.0)
                t2 = ptdpool.tile([P, 512], FP8, tag="ptd2")
                nc.vector.memset(t2[:, 0:P], 0.0)
            # prewarm ScalarE's exp table set during the QKV phase so the
            # first attention exp doesn't pay the ACT_TABLE_LOAD
            warm_sb = wpool.tile([1, 2], F32, tag="warm")
            nc.vector.memset(warm_sb[:], 0.0)
            nc.scalar.activation(warm_sb[:, 1:2], warm_sb[:, 0:1],
                                 mybir.ActivationFunctionType.Exp)

            # ---- phase 1: Q^T, K^T  (col-tile m: 0-2 = Q pairs, 3-5 = K pairs)
            qt_t, kt_t = [None] * NP, [None] * NP

            def emit_qk(m, n):
                # one quarter (512 cols) of one QKV^T col-tile — a ~1.3us PE
                # unit so it can slot into attention pipeline gaps
                if n == 0:
                    dst = qkvpool.tile([P, S], BF16, tag=f"qkvT{m}", name=f"qkvT{m}")
                    if m < NP:
                        qt_t[m] = dst
                    else:
                        kt_t[m - NP] = dst
                else:
                    dst = qt_t[m] if m < NP else kt_t[m - NP]
                ps = scpool.tile([P, 1024], F32, tag="sc")
                for t in range(NDT):
                    nc.tensor.matmul(
                        ps[:, 0:QT],
                        lhsT=wak[t][:, bass.ts(m, P)],
                        rhs=xk[t][:, bass.ts(n, QT)],
                        start=(t == 0),
                        stop=(t == NDT - 1),
                    )
                nc.vector.tensor_scalar_add(
                    dst[:, bass.ts(n, QT)],
                    ps[:, 0:QT],
                    ba_sb[:, m : m + 1],
                )

            # V j-pair tiles (fp8): v_t[u] holds s-tiles (2u+1, 2u) in free
            # slots (t=0 -> odd, t=1 -> even) to match the score-pair order.
            # u 0/1 also get bf16 copies: q-tile i=0 (few attended keys, so
            # quantization noise doesn't average out) runs a bf16 AV path.
            v_t = [None] * (S // P // 2)
            v8_t = [None, None]

            def emit_v(s):
                ps = scpool.tile([P, 1024], F32, tag="sc")
                for t in range(NDT):
                    nc.tensor.matmul(
                        ps[:, 0:DL],
                        lhsT=xk[t][:, bass.ts(s, P)],
                        rhs=wak[t][:, 2 * DL : 3 * DL],
                        start=(t == 0),
                        stop=(t == NDT - 1),
                    )
                u, odd = divmod(s, 2)
                if v_t[u] is None:
                    v_t[u] = qkvpool.tile([P, HL * 2 * HD], FP8, tag=f"v{u}", name=f"v{u}")
                vt4 = v_t[u][:].rearrange("p (h t x) -> p h t x", h=HL, t=2)
                nc.vector.tensor_add(
                    vt4[:, :, 0 if odd else 1, :],
                    ps[:, 0:DL].rearrange("p (h x) -> p h x", h=HL),
                    bv_sb[:].rearrange("p (h x) -> p h x", h=HL),
                )
                if u < 2:
                    if v8_t[u] is None:
                        v8_t[u] = qkvpool.tile([P, HL * 2 * HD], BF16, tag=f"v8{u}", name=f"v8{u}")
                    v8t4 = v8_t[u][:].rearrange("p (h t x) -> p h t x", h=HL, t=2)
                    nc.vector.tensor_add(
                        v8t4[:, :, 0 if odd else 1, :],
                        ps[:, 0:DL].rearrange("p (h x) -> p h x", h=HL),
                        bv_sb[:].rearrange("p (h x) -> p h x", h=HL),
                    )

            yt_t = [None] * NP

            def emit_attn(i, p, fillers=()):
                hA, hB = 2 * p, 2 * p + 1
                avA = avpool.tile([HD, 1024], F32, tag="avA")
                avB = avpool.tile([HD, 1024], F32, tag="avB")
                n_pairs = 2 * i + 2
                scs = {}

                # score blocks per pair u: (j, col_off, q0, n); block order is
                # (odd j, even j) to match the V-pair layout and to make the
                # diagonal pairs' valid region contiguous
                def blocks_of(u):
                    if u < 2 * i:
                        return [(2 * u + 1, 0, i * QT, QT), (2 * u, QT, i * QT, QT)]
                    if u == 2 * i:
                        return [(4 * i + 1, P, i * QT + P, 384), (4 * i, QT, i * QT, QT)]
                    return [(4 * i + 3, P, i * QT + 384, P), (4 * i + 2, 256, i * QT + 256, 256)]

                def emit_scores(u):
                    scA = scpool.tile([P, 1024], F32, tag="sc")
                    scB = scpool.tile([P, 1024], F32, tag="sc")
                    for j, off, q0, n in blocks_of(u):
                        for sc, lo in ((scA, 0), (scB, HD)):
                            nc.tensor.matmul(
                                sc[:, off : off + n],
                                lhsT=kt_t[p][lo : lo + HD, bass.ts(j, P)],
                                rhs=qt_t[p][lo : lo + HD, q0 : q0 + n],
                                start=True, stop=True,
                            )
                    scs[u] = (scA, scB)

                def process(u):
                    scA, scB = scs.pop(u)
                    bf = i == 0  # bf16 AV path for the first q-tile
                    PDT = BF16 if bf else FP8
                    mtile = mskb_sb if bf else msk_sb
                    if u < 2 * i:
                        e0, e1 = 0, 1024
                        ptA = ptpool.tile([P, 1024], PDT, tag="pt")
                        ptB = ptpool.tile([P, 1024], PDT, tag="pt")
                        masks = []
                        qoff, qn = 0, QT
                    elif u == 2 * i:
                        e0, e1 = P, 1024
                        ptA = ptdpool.tile([P, 1024], PDT, tag="ptd1b" if bf else "ptd1")
                        ptB = ptdpool.tile([P, 1024], PDT, tag="ptd1b" if bf else "ptd1")
                        masks = [(P, 256), (QT, QT + P)]
                        qoff, qn = 0, QT
                    else:
                        e0, e1 = P, 512
                        ptA = ptdpool.tile([P, 512], PDT, tag="ptd2b" if bf else "ptd2")
                        ptB = ptdpool.tile([P, 512], PDT, tag="ptd2b" if bf else "ptd2")
                        masks = [(P, 256), (256, 384)]
                        qoff, qn = 256, 256
                    for sc, pt in ((scA, ptA), (scB, ptB)):
                        nc.scalar.activation(pt[:, e0:e1], sc[:, e0:e1],
                                             mybir.ActivationFunctionType.Exp, scale=0.125)
                        for m0, m1 in masks:
                            nc.vector.tensor_mul(pt[:, m0:m1], pt[:, m0:m1], mtile[:])
                    first, last = (u == 0), (u == n_pairs - 1)
                    if bf:
                        # plain bf16 matmuls over the exact valid block ranges
                        # (block order: j0-full first so `start` covers the
                        # whole region)
                        v8t4 = v8_t[u][:].rearrange("p (h t x) -> p h t x", h=HL, t=2)
                        if u == 0:
                            blks = [(1, QT, 0, QT), (0, P, P, 384)]
                        else:
                            blks = [(1, 256, 256, 256), (0, P, 384, P)]
                        for pt, av, h in ((ptA, avA, hA), (ptB, avB, hB)):
                            for bi, (t, off, ql, n) in enumerate(blks):
                                st = first and bi == 0
                                sp = last and bi == len(blks) - 1
                                nc.tensor.matmul(
                                    av[:, ql : ql + n],
                                    lhsT=v8t4[:, h, t, :], rhs=pt[:, off : off + n],
                                    start=st, stop=sp,
                                )
                                nc.tensor.matmul(
                                    av[:, QT + ql : QT + ql + n],
                                    lhsT=onesb_sb[:], rhs=pt[:, off : off + n],
                                    start=st, stop=sp,
                                )
                    else:
                        for pt, av, h in ((ptA, avA, hA), (ptB, avB, hB)):
                            rhs = pt[:, 0 : 2 * qn].rearrange("p (t n) -> p t n", t=2)
                            vt4 = v_t[u][:].rearrange("p (h t x) -> p h t x", h=HL, t=2)
                            nc.tensor.matmul(
                                av[:, qoff : qoff + qn], lhsT=vt4[:, h, :, :], rhs=rhs,
                                start=first, stop=last, perf_mode=DR,
                            )
                            nc.tensor.matmul(
                                av[:, QT + qoff : QT + qoff + qn], lhsT=ones2v, rhs=rhs,
                                start=first, stop=last, perf_mode=DR,
                            )

                emit_scores(0)
                for u in range(n_pairs):
                    if u + 1 < n_pairs:
                        emit_scores(u + 1)
                    process(u)
                    # PE filler work (QKV/V/proj/RS units) slotted into the
                    # Act-paced attention pipeline
                    if u < len(fillers):
                        for th in fillers[u]:
                            th()

                # normalize: n is partition-replicated in av[:, 512:1024], so
                # ytp = y_u * recip(n) elementwise; head B bounced via DMA to
                # reach partitions 64:128
                ytp = ytpool.tile([P, QT], BF16, tag=f"yt{p}")
                rbA = nrmpool.tile([HD, QT], F32, tag="rbA")
                nc.vector.reciprocal(rbA[:], avA[:, QT:])
                nc.vector.tensor_mul(ytp[0:HD, :], avA[:, 0:QT], rbA[:])
                rbB = nrmpool.tile([HD, QT], F32, tag="rbB")
                nc.vector.reciprocal(rbB[:], avB[:, QT:])
                tmpB = nrmpool.tile([HD, QT], BF16, tag="tmpB")
                nc.vector.tensor_mul(tmpB[:], avB[:, 0:QT], rbB[:])
                nc.sync.dma_start(ytp[HD:P, :], tmpB[:])
                yt_t[p] = ytp

            # proj consumes the i-th q-tile's yt (stashed per i so filler-
            # scheduled proj units read the right generation)
            yt_gen = [None] * NI

            def emit_proj_ss(i, ss):
                yts = yt_gen[i]
                ps = scpool.tile([P, 1024], F32, tag="sc")
                for p in range(NP):
                    nc.tensor.matmul(
                        ps[:, 0:512],
                        lhsT=yts[p][:, bass.ts(ss, P)],
                        rhs=wpp[p][:, 0:512],
                        start=(p == 0), stop=(p == NP - 1),
                    )
                    nc.tensor.matmul(
                        ps[:, 512:768],
                        lhsT=yts[p][:, bass.ts(ss, P)],
                        rhs=wpp[p][:, 512:768],
                        start=(p == 0), stop=(p == NP - 1),
                    )
                ob = obpool.tile([P, D], F32, tag="ob")
                nc.vector.tensor_add(ob[:], ps[:, 0:D], bp_sb[:])
                row0 = i * QT + ss * P
                for c, (r0, r1) in enumerate(chunk_rows):
                    if r0 <= row0 < r1:
                        nc.sync.dma_start(ar_ins[c][row0 - r0 : row0 - r0 + P, :], ob[:])

            def emit_rs(c):
                # ReduceScatter(add) over the core pair: even core receives the
                # summed first half of the chunk, odd core the second half;
                # result bounced through SBUF into `out` (host stitches)
                nc.gpsimd.collective_compute(
                    "ReduceScatter",
                    mybir.AluOpType.add,
                    replica_groups=[[0, 1], [2, 3], [4, 5], [6, 7]],
                    ins=[ar_ins[c][:, :].opt()],
                    outs=[rs_outs[c][:, :].opt()],
                )
                base = [0, 512, 768][c]
                nrows = (chunk_rows[c][1] - chunk_rows[c][0]) // 2
                for r in range(0, nrows, P):
                    oc = obpool.tile([P, D], F32, tag="oc")
                    nc.sync.dma_start(oc[:], rs_outs[c][r : r + P, :])
                    nc.sync.dma_start(out[base + r : base + r + P, :], oc[:])

            # ---- main interleaved schedule ----
            # attention for q-tile i only needs Q/K col-quarters n <= i and V
            # s-tiles < 4(i+1), so QKV/V/proj/RS units slot into the Act-paced
            # attention pipeline as per-group PE fillers (engine execution
            # follows emission order, so placement here IS the schedule)
            def Q(m, n):
                return lambda: emit_qk(m, n)

            def V(s):
                return lambda: emit_v(s)

            def PJ(i, ss):
                return lambda: emit_proj_ss(i, ss)

            def RS(c):
                return lambda: emit_rs(c)

            for _rep in range(REPEAT):
                emit_qk(0, 0)
                emit_qk(3, 0)
                for s in range(4):
                    emit_v(s)
                emit_attn(0, 0, [[Q(1, 0)], [Q(4, 0)]])
                emit_attn(0, 1, [[Q(2, 0)], [Q(5, 0)]])
                emit_attn(0, 2, [[Q(0, 1)], [Q(3, 1)]])
                yt_gen[0] = list(yt_t)
                emit_attn(1, 0, [[V(4), V(5)], [V(6), V(7)], [Q(1, 1)], [Q(4, 1)]])
                emit_attn(1, 1, [[Q(2, 1)], [Q(5, 1)], [PJ(0, 0)], [PJ(0, 1)]])
                emit_attn(1, 2, [[Q(0, 2)], [Q(3, 2)], [PJ(0, 2)], [PJ(0, 3)]])
                yt_gen[1] = list(yt_t)
                emit_attn(2, 0, [[V(8), V(9)], [V(10), V(11)], [Q(1, 2)], [Q(4, 2)], [PJ(1, 0)], [PJ(1, 1)]])
                emit_attn(2, 1, [[Q(2, 2)], [Q(5, 2)], [PJ(1, 2)], [PJ(1, 3)], [RS(0)], [Q(0, 3)]])
                emit_attn(2, 2, [[Q(3, 3)], [Q(1, 3)], [Q(4, 3)], [Q(2, 3)], [Q(5, 3)], []])
                yt_gen[2] = list(yt_t)
                emit_attn(3, 0, [[V(12), V(13)], [V(14), V(15)], [PJ(2, 0)], [PJ(2, 1)], [PJ(2, 2)], [PJ(2, 3)], [RS(1)], []])
                emit_attn(3, 1, [])
                emit_attn(3, 2, [])
                yt_gen[3] = list(yt_t)
                for ss in range(4):
                    emit_proj_ss(3, ss)
                emit_rs(2)

    _legalize_waits(nc)
    return nc


_NC_CACHE = {}


def _get_nc():
    if "nc" not in _NC_CACHE:
        _NC_CACHE["nc"] = _build()
    return _NC_CACHE["nc"]


def _prep_inputs(x, W_attn, b_attn, W_proj, b_proj):
    bf = ml_dtypes.bfloat16
    fp8 = ml_dtypes.float8_e4m3
    x = np.asarray(x, np.float32)
    W_attn = np.asarray(W_attn, np.float32)
    b_attn = np.asarray(b_attn, np.float32)
    W_proj = np.asarray(W_proj, np.float32)
    b_proj = np.asarray(b_proj, np.float32)

    # stair mask for the 128-col diagonal of each diag j-block
    mask = (np.arange(P)[None, :] >= np.arange(P)[:, None]).astype(fp8)

    in_maps = []
    for c in range(N_CORES):
        b, g = divmod(c, 2)
        cols = slice(DL * g, DL * g + DL)
        xT = np.ascontiguousarray(x[b].T).astype(bf)
        wa = np.concatenate(
            [W_attn[:, 0:D][:, cols], W_attn[:, D : 2 * D][:, cols], W_attn[:, 2 * D :][:, cols]],
            axis=1,
        ).astype(bf)
        ba_sl = np.concatenate(
            [b_attn[0:D][cols], b_attn[D : 2 * D][cols], b_attn[2 * D :][cols]]
        ).astype(np.float32)
        ba2 = np.ascontiguousarray(ba_sl[: 2 * DL].reshape(6, P).T)
        ba9 = np.zeros((P, 9), np.float32)
        ba9[:, :6] = ba2
        bv_b = np.ascontiguousarray(np.broadcast_to(ba_sl[2 * DL :], (P, DL))).astype(np.float32)
        wp_c = np.ascontiguousarray(W_proj[cols, :]).astype(bf)
        bp_full = b_proj if g == 0 else np.zeros_like(b_proj)
        bp_b = np.ascontiguousarray(np.broadcast_to(bp_full, (P, D))).astype(np.float32)
        in_maps.append(
            {
                "xT": xT,
                "wa": wa,
                "ba": ba9,
                "bv": bv_b,
                "wp": wp_c,
                "bp": bp_b,
                "msk": mask,
            }
        )
    return in_maps


def kernel(x, W_attn, b_attn, W_proj, b_proj):
    in_maps = _prep_inputs(x, W_attn, b_attn, W_proj, b_proj)
    nc = _get_nc()
    res = run_bass_kernel_spmd(nc, in_maps, list(range(N_CORES)))
    # stitch: chunk c (global rows r0:r1) -> even core's out rows
    # [base:base+h] = summed r0:r0+h, odd core's = r0+h:r1
    outs = []
    for b in range(B):
        rows = []
        for c, (r0, r1) in enumerate([(0, 1024), (1024, 1536), (1536, 2048)]):
            h = (r1 - r0) // 2
            base = [0, 512, 768][c]
            rows.append(res.results[2 * b]["out"][base : base + h])
            rows.append(res.results[2 * b + 1]["out"][base : base + h])
        outs.append(np.concatenate(rows, axis=0))
    return np.stack(outs).astype(np.float32)


# revision 26
# speedup vs baseline: 1.1094x; 1.0304x over previous
"""Causal multi-head attention block (GPT-style) for Trainium2, 8 NeuronCores.

Problem: x[4,2048,768] -> qkv = x@W_attn+b_attn -> 12-head causal attention
         -> y@W_proj+b_proj -> out[4,2048,768]   (fp32 I/O)

Sharding: 4 batches x 2 head-groups (6 heads each). c_attn column-sharded,
c_proj row-sharded over head groups; ReduceScatter(add) over core pairs after
c_proj (even core keeps summed rows 0:1024, odd core rows 1024:2048; host
stitches). Core c = 2*b + g handles batch b, heads 6g..6g+5.

Per-core kernel:
  1. QKV^T = Wa_g^T @ x^T (bf16 matmuls): Q^T,K^T [384,2048] pair-packed
     col-tile-major; V [2048,384] stored fp8e4 in j-PAIR layout
     [128, head, 2, 64] with pair order (odd s-tile, even s-tile).
  2. Flash-style causal attention in transposed-score orientation:
     S^T[k,q] blocks via row-packed pair matmuls (K=64 head A rows 0-63 /
     head B rows 64-127) in bf16, j-tiles grouped in PAIRS with block
     order (odd j, even j) so diagonal pairs have contiguous valid
     regions; exp on ScalarE (PSUM->SBUF fp8e4, scale=1/8); stair
     masking via 128-col mask multiplies (pad cols are startup-zeroed
     pool buffers); y_u^T and the softmax normalizer n via fp8
     DoubleRow matmuls (two j-tiles contracted per instruction, 0.5
     cycles/row): lhsT = V-pair [128,2,64] / replicated-ones [128,2,64],
     accumulating into av[64, 0:512]=y_u, av[64, 512:1024]=n-broadcast.
  3. Normalize elementwise: ytp = y_u * reciprocal(n) (n already
     partition-replicated by the ones-matmul, so no broadcast needed).
  4. proj: out_partial[s,768] = sum_pairs yT_pair^T @ Wp_pair, + b_proj.
  5. Three chunked ReduceScatters over {2b,2b+1} ([0:1024] after i=1,
     [1024:1536] after i=2, [1536:2048] at the end), each ~15us modeled,
     results bounced rs_out -> SBUF -> out.

The walrus build here allows only one sync-wait per instruction; a post-pass
(legalize_waits) hoists extra waits onto single-wait NOPs.
"""
import numpy as np
import ml_dtypes

import concourse.bass as bass
import concourse.tile as tile
from concourse import mybir
from concourse.bass_utils import run_bass_kernel_spmd
from concourse import mybir as mb

BF16 = mybir.dt.bfloat16
F32 = mybir.dt.float32
FP8 = mybir.dt.float8e4
DR = mybir.MatmulPerfMode.DoubleRow

B, S, D = 4, 2048, 768
H, HD = 12, 64
G = 2                 # head groups
HL = H // G           # heads per core = 6
DL = HL * HD          # local head dims = 384
NP = HL // 2          # head pairs per core = 3
P = 128
QT = 512              # q tile
NI = S // QT          # 4 q tiles
NDT = D // P          # 6 D tiles
N_CORES = 8
REPEAT = 1


def _legalize_waits(nc):
    n_split = 0
    for f in nc.m.functions:
        for bb in f.blocks:
            insts = list(bb.instructions)
            out = []
            changed = False
            for inst in insts:
                si = inst.sync_info
                if si is not None:
                    waits = list(si.on_wait)
                    if len(waits) > 1:
                        for w in waits[:-1]:
                            nop = mb.InstNoOp(name=f"I-wsplit-{nc.next_id()}", ins=[], outs=[])
                            nop.engine = inst.engine
                            nop.sync_info = mb.SyncInfo(on_wait=[w], on_update=[])
                            out.append(nop)
                            n_split += 1
                        inst.sync_info = mb.SyncInfo(on_wait=[waits[-1]], on_update=list(si.on_update))
                        changed = True
                out.append(inst)
            if changed:
                bb.instructions = out
    return n_split


def _build():
    nc = bass.Bass("TRN2", target_bir_lowering=False, debug=False, num_devices=N_CORES)

    xT = nc.dram_tensor("xT", [D, S], BF16, kind="ExternalInput").ap()
    wa = nc.dram_tensor("wa", [D, 3 * DL], BF16, kind="ExternalInput").ap()
    ba = nc.dram_tensor("ba", [P, 9], F32, kind="ExternalInput").ap()
    bv = nc.dram_tensor("bv", [P, DL], F32, kind="ExternalInput").ap()
    wp = nc.dram_tensor("wp", [DL, D], BF16, kind="ExternalInput").ap()
    bp = nc.dram_tensor("bp", [P, D], F32, kind="ExternalInput").ap()
    msk = nc.dram_tensor("msk", [P, P], FP8, kind="ExternalInput").ap()
    out = nc.dram_tensor("out", [S // 2, D], F32, kind="ExternalOutput").ap()
    # ReduceScatter chunks: separate in-tensors so proj writes emitted after
    # an RS carry no false WAR dep; internal outs (collectives can't write IO)
    chunk_rows = [(0, 1024), (1024, 1536), (1536, 2048)]
    ar_ins = [
        nc.dram_tensor(f"ar_in{c}", [r1 - r0, D], F32).ap()
        for c, (r0, r1) in enumerate(chunk_rows)
    ]
    rs_outs = [
        nc.dram_tensor(f"rs_out{c}", [(r1 - r0) // 2, D], F32).ap()
        for c, (r0, r1) in enumerate(chunk_rows)
    ]

    with tile.TileContext(nc) as tc:
        with (
            tc.tile_pool(name="wgt", bufs=1) as wpool,
            tc.tile_pool(name="qkv", bufs=1) as qkvpool,
            tc.tile_pool(name="pt", bufs=4) as ptpool,
            tc.tile_pool(name="ptd", bufs=2) as ptdpool,
            tc.tile_pool(name="yt", bufs=2) as ytpool,
            tc.tile_pool(name="nrm", bufs=3) as nrmpool,
            tc.tile_pool(name="ob", bufs=3) as obpool,
            tc.tile_pool(name="scp", bufs=2, space="PSUM") as scpool,
            tc.tile_pool(name="avp", bufs=1, space="PSUM") as avpool,
        ):
            # ---- phase 0: load weights/constants ----
            # small constants (ba: gates the Q/K bias adds and thus every
            # score/exp; msk: gates the first masks) go FIRST on the
            # serialized DMA device, then wa/x tiles interleaved so the first
            # QKV matmuls start as early as possible.
            ba_sb = wpool.tile([P, 9], F32, tag="ba")
            nc.sync.dma_start(ba_sb[:], ba[:])
            msk_sb = wpool.tile([P, P], FP8, tag="msk")
            nc.sync.dma_start(msk_sb[:], msk[:])
            wak, xk = [], []
            for t in range(NDT):
                wt_sb = wpool.tile([P, 3 * DL], BF16, tag=f"wak{t}")
                nc.sync.dma_start(wt_sb[:], wa[bass.ts(t, P), :])
                wak.append(wt_sb)
                xt_sb = wpool.tile([P, S], BF16, tag=f"xk{t}")
                nc.sync.dma_start(xt_sb[:, 0 : S // 2], xT[bass.ts(t, P), 0 : S // 2])
                xk.append(xt_sb)
                if t == 0:
                    bv_sb = wpool.tile([P, DL], F32, tag="bv")
                    nc.sync.dma_start(bv_sb[:], bv[:])
            for t in range(NDT):
                nc.sync.dma_start(xk[t][:, S // 2 :], xT[bass.ts(t, P), S // 2 :])
            wpp = []
            for p in range(NP):
                wp_sb = wpool.tile([P, D], BF16, tag=f"wpp{p}")
                nc.sync.dma_start(wp_sb[:], wp[bass.ts(p, P), :])
                wpp.append(wp_sb)
            bp_sb = wpool.tile([P, D], F32, tag="bp")
            nc.sync.dma_start(bp_sb[:], bp[:])
            mskb_sb = wpool.tile([P, P], BF16, tag="mskb")
            nc.vector.tensor_copy(mskb_sb[:], msk_sb[:])
            ones2_sb = wpool.tile([P, P], FP8, tag="ones2")
            nc.vector.memset(ones2_sb[:], 1.0)
            ones2v = ones2_sb[:].rearrange("p (t m) -> p t m", t=2)
            onesb_sb = wpool.tile([P, HD], BF16, tag="onesb")
            nc.vector.memset(onesb_sb[:], 1.0)
            # pre-zero the pad region [0:128] of every diag-pt pool buffer;
            # steady state keeps it zero (only read afterwards)
            for _b in range(2):
                t1 = ptdpool.tile([P, 1024], FP8, tag="ptd1")
                nc.vector.memset(t1[:, 0:P], 0.0)
                t2 = ptdpool.tile([P, 512], FP8, tag="ptd2")
                nc.vector.memset(t2[:, 0:P], 0.0)
            # prewarm ScalarE's exp table set during the QKV phase so the
            # first attention exp doesn't pay the ACT_TABLE_LOAD
            warm_sb = wpool.tile([1, 2], F32, tag="warm")
            nc.vector.memset(warm_sb[:], 0.0)
            nc.scalar.activation(warm_sb[:, 1:2], warm_sb[:, 0:1],
                                 mybir.ActivationFunctionType.Exp)

            # ---- phase 1: Q^T, K^T  (col-tile m: 0-2 = Q pairs, 3-5 = K pairs)
            qt_t, kt_t = [None] * NP, [None] * NP

            def emit_qk(m, n):
                # one quarter (512 cols) of one QKV^T col-tile — a ~1.3us PE
                # unit so it can slot into attention pipeline gaps
                if n == 0:
                    dst = qkvpool.tile([P, S], BF16, tag=f"qkvT{m}", name=f"qkvT{m}")
                    if m < NP:
                        qt_t[m] = dst
                    else:
                        kt_t[m - NP] = dst
                else:
                    dst = qt_t[m] if m < NP else kt_t[m - NP]
                ps = scpool.tile([P, 1024], F32, tag="sc")
                for t in range(NDT):
                    nc.tensor.matmul(
                        ps[:, 0:QT],
                        lhsT=wak[t][:, bass.ts(m, P)],
                        rhs=xk[t][:, bass.ts(n, QT)],
                        start=(t == 0),
                        stop=(t == NDT - 1),
                    )
                nc.vector.tensor_scalar_add(
                    dst[:, bass.ts(n, QT)],
                    ps[:, 0:QT],
                    ba_sb[:, m : m + 1],
                )

            # V j-pair tiles (fp8): v_t[u] holds s-tiles (2u+1, 2u) in free
            # slots (t=0 -> odd, t=1 -> even) to match the score-pair order.
            # u 0/1 also get bf16 copies: q-tile i=0 (few attended keys, so
            # quantization noise doesn't average out) runs a bf16 AV path.
            v_t = [None] * (S // P // 2)
            v8_t = [None, None]

            def emit_v(s):
                ps = scpool.tile([P, 1024], F32, tag="sc")
                for t in range(NDT):
                    nc.tensor.matmul(
                        ps[:, 0:DL],
                        lhsT=xk[t][:, bass.ts(s, P)],
                        rhs=wak[t][:, 2 * DL : 3 * DL],
                        start=(t == 0),
                        stop=(t == NDT - 1),
                    )
                u, odd = divmod(s, 2)
                if v_t[u] is None:
                    v_t[u] = qkvpool.tile([P, HL * 2 * HD], FP8, tag=f"v{u}", name=f"v{u}")
                vt4 = v_t[u][:].rearrange("p (h t x) -> p h t x", h=HL, t=2)
                nc.vector.tensor_add(
                    vt4[:, :, 0 if odd else 1, :],
                    ps[:, 0:DL].rearrange("p (h x) -> p h x", h=HL),
                    bv_sb[:].rearrange("p (h x) -> p h x", h=HL),
                )
                if u < 2:
                    if v8_t[u] is None:
                        v8_t[u] = qkvpool.tile([P, HL * 2 * HD], BF16, tag=f"v8{u}", name=f"v8{u}")
                    v8t4 = v8_t[u][:].rearrange("p (h t x) -> p h t x", h=HL, t=2)
                    nc.vector.tensor_add(
                        v8t4[:, :, 0 if odd else 1, :],
                        ps[:, 0:DL].rearrange("p (h x) -> p h x", h=HL),
                        bv_sb[:].rearrange("p (h x) -> p h x", h=HL),
                    )

            yt_t = [None] * NP

            def emit_attn(i, p, fillers=()):
                hA, hB = 2 * p, 2 * p + 1
                avA = avpool.tile([HD, 1024], F32, tag="avA")
                avB = avpool.tile([HD, 1024], F32, tag="avB")
                n_pairs = 2 * i + 2
                scs = {}

                # score blocks per pair u: (j, col_off, q0, n); block order is
                # (odd j, even j) to match the V-pair layout and to make the
                # diagonal pairs' valid region contiguous
                def blocks_of(u):
                    if u < 2 * i:
                        return [(2 * u + 1, 0, i * QT, QT), (2 * u, QT, i * QT, QT)]
                    if u == 2 * i:
                        return [(4 * i + 1, P, i * QT + P, 384), (4 * i, QT, i * QT, QT)]
                    return [(4 * i + 3, P, i * QT + 384, P), (4 * i + 2, 256, i * QT + 256, 256)]

                def emit_scores(u):
                    scA = scpool.tile([P, 1024], F32, tag="sc")
                    scB = scpool.tile([P, 1024], F32, tag="sc")
                    for j, off, q0, n in blocks_of(u):
                        for sc, lo in ((scA, 0), (scB, HD)):
                            nc.tensor.matmul(
                                sc[:, off : off + n],
                                lhsT=kt_t[p][lo : lo + HD, bass.ts(j, P)],
                                rhs=qt_t[p][lo : lo + HD, q0 : q0 + n],
                                start=True, stop=True,
                            )
                    scs[u] = (scA, scB)

                def process(u):
                    scA, scB = scs.pop(u)
                    bf = i == 0  # bf16 AV path for the first q-tile
                    PDT = BF16 if bf else FP8
                    mtile = mskb_sb if bf else msk_sb
                    if u < 2 * i:
                        e0, e1 = 0, 1024
                        ptA = ptpool.tile([P, 1024], PDT, tag="pt")
                        ptB = ptpool.tile([P, 1024], PDT, tag="pt")
                        masks = []
                        qoff, qn = 0, QT
                    elif u == 2 * i:
                        e0, e1 = P, 1024
                        ptA = ptdpool.tile([P, 1024], PDT, tag="ptd1b" if bf else "ptd1")
                        ptB = ptdpool.tile([P, 1024], PDT, tag="ptd1b" if bf else "ptd1")
                        masks = [(P, 256), (QT, QT + P)]
                        qoff, qn = 0, QT
                    else:
                        e0, e1 = P, 512
                        ptA = ptdpool.tile([P, 512], PDT, tag="ptd2b" if bf else "ptd2")
                        ptB = ptdpool.tile([P, 512], PDT, tag="ptd2b" if bf else "ptd2")
                        masks = [(P, 256), (256, 384)]
                        qoff, qn = 256, 256
                    for sc, pt in ((scA, ptA), (scB, ptB)):
                        nc.scalar.activation(pt[:, e0:e1], sc[:, e0:e1],
                                             mybir.ActivationFunctionType.Exp, scale=0.125)
                        for m0, m1 in masks:
                            nc.vector.tensor_mul(pt[:, m0:m1], pt[:, m0:m1], mtile[:])
                    first, last = (u == 0), (u == n_pairs - 1)
                    if bf:
                        # plain bf16 matmuls over the exact valid block ranges
                        # (block order: j0-full first so `start` covers the
                        # whole region)
                        v8t4 = v8_t[u][:].rearrange("p (h t x) -> p h t x", h=HL, t=2)
                        if u == 0:
                            blks = [(1, QT, 0, QT), (0, P, P, 384)]
                        else:
                            blks = [(1, 256, 256, 256), (0, P, 384, P)]
                        for pt, av, h in ((ptA, avA, hA), (ptB, avB, hB)):
                            for bi, (t, off, ql, n) in enumerate(blks):
                                st = first and bi == 0
                                sp = last and bi == len(blks) - 1
                                nc.tensor.matmul(
                                    av[:, ql : ql + n],
                                    lhsT=v8t4[:, h, t, :], rhs=pt[:, off : off + n],
                                    start=st, stop=sp,
                                )
                                nc.tensor.matmul(
                                    av[:, QT + ql : QT + ql + n],
                                    lhsT=onesb_sb[:], rhs=pt[:, off : off + n],
                                    start=st, stop=sp,
                                )
                    else:
                        for pt, av, h in ((ptA, avA, hA), (ptB, avB, hB)):
                            rhs = pt[:, 0 : 2 * qn].rearrange("p (t n) -> p t n", t=2)
                            vt4 = v_t[u][:].rearrange("p (h t x) -> p h t x", h=HL, t=2)
                            nc.tensor.matmul(
                                av[:, qoff : qoff + qn], lhsT=vt4[:, h, :, :], rhs=rhs,
                                start=first, stop=last, perf_mode=DR,
                            )
                            nc.tensor.matmul(
                                av[:, QT + qoff : QT + qoff + qn], lhsT=ones2v, rhs=rhs,
                                start=first, stop=last, perf_mode=DR,
                            )

                emit_scores(0)
                for u in range(n_pairs):
                    if u + 1 < n_pairs:
                        emit_scores(u + 1)
                    process(u)
                    # PE filler work (QKV/V/proj/RS units) slotted into the
                    # Act-paced attention pipeline
                    if u < len(fillers):
                        for th in fillers[u]:
                            th()

                # normalize: n is partition-replicated in av[:, 512:1024], so
                # ytp = y_u * recip(n) elementwise; head B bounced via DMA to
                # reach partitions 64:128
                ytp = ytpool.tile([P, QT], BF16, tag=f"yt{p}")
                rbA = nrmpool.tile([HD, QT], F32, tag="rbA")
                nc.vector.reciprocal(rbA[:], avA[:, QT:])
                nc.vector.tensor_mul(ytp[0:HD, :], avA[:, 0:QT], rbA[:])
                rbB = nrmpool.tile([HD, QT], F32, tag="rbB")
                nc.vector.reciprocal(rbB[:], avB[:, QT:])
                tmpB = nrmpool.tile([HD, QT], BF16, tag="tmpB")
                nc.vector.tensor_mul(tmpB[:], avB[:, 0:QT], rbB[:])
                nc.sync.dma_start(ytp[HD:P, :], tmpB[:])
                yt_t[p] = ytp

            # proj consumes the i-th q-tile's yt (stashed per i so filler-
            # scheduled proj units read the right generation)
            yt_gen = [None] * NI

            def emit_proj_ss(i, ss):
                yts = yt_gen[i]
                ps = scpool.tile([P, 1024], F32, tag="sc")
                for p in range(NP):
                    nc.tensor.matmul(
                        ps[:, 0:512],
                        lhsT=yts[p][:, bass.ts(ss, P)],
                        rhs=wpp[p][:, 0:512],
                        start=(p == 0), stop=(p == NP - 1),
                    )
                    nc.tensor.matmul(
                        ps[:, 512:768],
                        lhsT=yts[p][:, bass.ts(ss, P)],
                        rhs=wpp[p][:, 512:768],
                        start=(p == 0), stop=(p == NP - 1),
                    )
                ob = obpool.tile([P, D], F32, tag="ob")
                nc.vector.tensor_add(ob[:], ps[:, 0:D], bp_sb[:])
                row0 = i * QT + ss * P
                for c, (r0, r1) in enumerate(chunk_rows):
                    if r0 <= row0 < r1:
                        nc.sync.dma_start(ar_ins[c][row0 - r0 : row0 - r0 + P, :], ob[:])

            def emit_rs(c):
                # ReduceScatter(add) over the core pair: even core receives the
                # summed first half of the chunk, odd core the second half;
                # result bounced through SBUF into `out` (host stitches)
                nc.gpsimd.collective_compute(
                    "ReduceScatter",
                    mybir.AluOpType.add,
                    replica_groups=[[0, 1], [2, 3], [4, 5], [6, 7]],
                    ins=[ar_ins[c][:, :].opt()],
                    outs=[rs_outs[c][:, :].opt()],
                )
                base = [0, 512, 768][c]
                nrows = (chunk_rows[c][1] - chunk_rows[c][0]) // 2
                for r in range(0, nrows, P):
                    oc = obpool.tile([P, D], F32, tag="oc")
                    nc.sync.dma_start(oc[:], rs_outs[c][r : r + P, :])
                    nc.sync.dma_start(out[base + r : base + r + P, :], oc[:])

            # ---- main interleaved schedule ----
            # attention for q-tile i only needs Q/K col-quarters n <= i and V
            # s-tiles < 4(i+1), so QKV/V/proj/RS units slot into the Act-paced
            # attention pipeline as per-group PE fillers (engine execution
            # follows emission order, so placement here IS the schedule)
            def Q(m, n):
                return lambda: emit_qk(m, n)

            def V(s):
                return lambda: emit_v(s)

            def PJ(i, ss):
                return lambda: emit_proj_ss(i, ss)

            def RS(c):
                return lambda: emit_rs(c)

            for _rep in range(REPEAT):
                emit_qk(0, 0)
                emit_qk(3, 0)
                for s in range(4):
                    emit_v(s)
                emit_attn(0, 0, [[Q(1, 0)], [Q(4, 0)]])
                emit_attn(0, 1, [[Q(2, 0)], [Q(5, 0)]])
                emit_attn(0, 2, [[Q(0, 1)], [Q(3, 1)]])
                yt_gen[0] = list(yt_t)
                emit_attn(1, 0, [[V(4), V(5)], [V(6), V(7)], [Q(1, 1)], [Q(4, 1)]])
                emit_attn(1, 1, [[Q(2, 1)], [Q(5, 1)], [PJ(0, 0)], [PJ(0, 1)]])
                emit_attn(1, 2, [[Q(0, 2)], [Q(3, 2)], [PJ(0, 2)], [PJ(0, 3)]])
                yt_gen[1] = list(yt_t)
                emit_attn(2, 0, [[V(8), V(9)], [V(10), V(11)], [Q(1, 2)], [Q(4, 2)], [PJ(1, 0)], [PJ(1, 1)]])
                emit_attn(2, 1, [[Q(2, 2)], [Q(5, 2)], [PJ(1, 2)], [PJ(1, 3)], [RS(0)], [Q(0, 3)]])
                emit_attn(2, 2, [[Q(3, 3)], [Q(1, 3)], [Q(4, 3)], [Q(2, 3)], [Q(5, 3)], []])
                yt_gen[2] = list(yt_t)
                emit_attn(3, 0, [[V(12), V(13)], [V(14), V(15)], [PJ(2, 0)], [PJ(2, 1)], [PJ(2, 2)], [PJ(2, 3)], [RS(1)], []])
                emit_attn(3, 1, [])
                emit_attn(3, 2, [])
                yt_gen[3] = list(yt_t)
                for ss in range(4):
                    emit_proj_ss(3, ss)
                emit_rs(2)

    _legalize_waits(nc)
    return nc


_NC_CACHE = {}


def _get_nc():
    if "nc" not in _NC_CACHE:
        _NC_CACHE["nc"] = _build()
    return _NC_CACHE["nc"]


def _prep_inputs(x, W_attn, b_attn, W_proj, b_proj):
    bf = ml_dtypes.bfloat16
    fp8 = ml_dtypes.float8_e4m3
    x = np.asarray(x, np.float32)
    W_attn = np.asarray(W_attn, np.float32)
    b_attn = np.asarray(b_attn, np.float32)
    W_proj = np.asarray(W_proj, np.float32)
    b_proj = np.asarray(b_proj, np.float32)

    # stair mask for the 128-col diagonal of each diag j-block
    mask = (np.arange(P)[None, :] >= np.arange(P)[:, None]).astype(fp8)

    in_maps = []
    for c in range(N_CORES):
        b, g = divmod(c, 2)
        cols = slice(DL * g, DL * g + DL)
        xT = np.ascontiguousarray(x[b].T).astype(bf)
        wa = np.concatenate(
            [W_attn[:, 0:D][:, cols], W_attn[:, D : 2 * D][:, cols], W_attn[:, 2 * D :][:, cols]],
            axis=1,
        ).astype(bf)
        ba_sl = np.concatenate(
            [b_attn[0:D][cols], b_attn[D : 2 * D][cols], b_attn[2 * D :][cols]]
        ).astype(np.float32)
        ba2 = np.ascontiguousarray(ba_sl[: 2 * DL].reshape(6, P).T)
        ba9 = np.zeros((P, 9), np.float32)
        ba9[:, :6] = ba2
        bv_b = np.ascontiguousarray(np.broadcast_to(ba_sl[2 * DL :], (P, DL))).astype(np.float32)
        wp_c = np.ascontiguousarray(W_proj[cols, :]).astype(bf)
        bp_full = b_proj if g == 0 else np.zeros_like(b_proj)
        bp_b = np.ascontiguousarray(np.broadcast_to(bp_full, (P, D))).astype(np.float32)
        in_maps.append(
            {
                "xT": xT,
                "wa": wa,
                "ba": ba9,
                "bv": bv_b,
                "wp": wp_c,
                "bp": bp_b,
                "msk": mask,
            }
        )
    return in_maps


def kernel(x, W_attn, b_attn, W_proj, b_proj):
    in_maps = _prep_inputs(x, W_attn, b_attn, W_proj, b_proj)
    nc = _get_nc()
    res = run_bass_kernel_spmd(nc, in_maps, list(range(N_CORES)))
    # stitch: chunk c (global rows r0:r1) -> even core's out rows
    # [base:base+h] = summed r0:r0+h, odd core's = r0+h:r1
    outs = []
    for b in range(B):
        rows = []
        for c, (r0, r1) in enumerate([(0, 1024), (1024, 1536), (1536, 2048)]):
            h = (r1 - r0) // 2
            base = [0, 512, 768][c]
            rows.append(res.results[2 * b]["out"][base : base + h])
            rows.append(res.results[2 * b + 1]["out"][base : base + h])
        outs.append(np.concatenate(rows, axis=0))
    return np.stack(outs).astype(np.float32)


# revision 35
# speedup vs baseline: 1.1105x; 1.0010x over previous
"""Causal multi-head attention block (GPT-style) for Trainium2, 8 NeuronCores.

Problem: x[4,2048,768] -> qkv = x@W_attn+b_attn -> 12-head causal attention
         -> y@W_proj+b_proj -> out[4,2048,768]   (fp32 I/O)

Sharding: 4 batches x 2 head-groups (6 heads each). c_attn column-sharded,
c_proj row-sharded over head groups; ReduceScatter(add) over core pairs after
c_proj (even core keeps summed rows 0:1024, odd core rows 1024:2048; host
stitches). Core c = 2*b + g handles batch b, heads 6g..6g+5.

Per-core kernel:
  1. QKV^T = Wa_g^T @ x^T (bf16 matmuls): Q^T,K^T [384,2048] pair-packed
     col-tile-major; V [2048,384] stored fp8e4 in j-PAIR layout
     [128, head, 2, 64] with pair order (odd s-tile, even s-tile).
  2. Flash-style causal attention in transposed-score orientation:
     S^T[k,q] blocks via row-packed pair matmuls (K=64 head A rows 0-63 /
     head B rows 64-127) in bf16, j-tiles grouped in PAIRS with block
     order (odd j, even j) so diagonal pairs have contiguous valid
     regions; exp on ScalarE (PSUM->SBUF fp8e4, scale=1/8); stair
     masking via 128-col mask multiplies (pad cols are startup-zeroed
     pool buffers); y_u^T and the softmax normalizer n via fp8
     DoubleRow matmuls (two j-tiles contracted per instruction, 0.5
     cycles/row): lhsT = V-pair [128,2,64] / replicated-ones [128,2,64],
     accumulating into av[64, 0:512]=y_u, av[64, 512:1024]=n-broadcast.
  3. Normalize elementwise: ytp = y_u * reciprocal(n) (n already
     partition-replicated by the ones-matmul, so no broadcast needed).
  4. proj: out_partial[s,768] = sum_pairs yT_pair^T @ Wp_pair, + b_proj.
  5. Three chunked ReduceScatters over {2b,2b+1} ([0:1024] after i=1,
     [1024:1536] after i=2, [1536:2048] at the end), each ~15us modeled,
     results bounced rs_out -> SBUF -> out.

The walrus build here allows only one sync-wait per instruction; a post-pass
(legalize_waits) hoists extra waits onto single-wait NOPs.
"""
import numpy as np
import ml_dtypes

import concourse.bass as bass
import concourse.tile as tile
from concourse import mybir
from concourse.bass_utils import run_bass_kernel_spmd
from concourse import mybir as mb

BF16 = mybir.dt.bfloat16
F32 = mybir.dt.float32
FP8 = mybir.dt.float8e4
DR = mybir.MatmulPerfMode.DoubleRow

B, S, D = 4, 2048, 768
H, HD = 12, 64
G = 2                 # head groups
HL = H // G           # heads per core = 6
DL = HL * HD          # local head dims = 384
NP = HL // 2          # head pairs per core = 3
P = 128
QT = 512              # q tile
NI = S // QT          # 4 q tiles
NDT = D // P          # 6 D tiles
N_CORES = 8
REPEAT = 1


def _legalize_waits(nc):
    n_split = 0
    for f in nc.m.functions:
        for bb in f.blocks:
            insts = list(bb.instructions)
            out = []
            changed = False
            for inst in insts:
                si = inst.sync_info
                if si is not None:
                    waits = list(si.on_wait)
                    if len(waits) > 1:
                        for w in waits[:-1]:
                            nop = mb.InstNoOp(name=f"I-wsplit-{nc.next_id()}", ins=[], outs=[])
                            nop.engine = inst.engine
                            nop.sync_info = mb.SyncInfo(on_wait=[w], on_update=[])
                            out.append(nop)
                            n_split += 1
                        inst.sync_info = mb.SyncInfo(on_wait=[waits[-1]], on_update=list(si.on_update))
                        changed = True
                out.append(inst)
            if changed:
                bb.instructions = out
    return n_split


def _build():
    nc = bass.Bass("TRN2", target_bir_lowering=False, debug=False, num_devices=N_CORES)

    xT = nc.dram_tensor("xT", [D, S], BF16, kind="ExternalInput").ap()
    wa = nc.dram_tensor("wa", [D, 3 * DL], BF16, kind="ExternalInput").ap()
    # fp8 pair-interleaved copies of x^T and the Q/K weight columns for the
    # DoubleRow QKV path (quarters n>=1; n=0 stays bf16 for early-row accuracy)
    xq = nc.dram_tensor("xq", [D // 2, 2 * S], FP8, kind="ExternalInput").ap()
    waq = nc.dram_tensor("waq", [D // 2, 2 * 2 * DL], FP8, kind="ExternalInput").ap()
    ba = nc.dram_tensor("ba", [P, 24], F32, kind="ExternalInput").ap()
    bv = nc.dram_tensor("bv", [P, DL], F32, kind="ExternalInput").ap()
    wp = nc.dram_tensor("wp", [DL, D], BF16, kind="ExternalInput").ap()
    bp = nc.dram_tensor("bp", [P, D], F32, kind="ExternalInput").ap()
    msk = nc.dram_tensor("msk", [P, P], FP8, kind="ExternalInput").ap()
    out = nc.dram_tensor("out", [S // 2, D], F32, kind="ExternalOutput").ap()
    # ReduceScatter chunks: separate in-tensors so proj writes emitted after
    # an RS carry no false WAR dep; internal outs (collectives can't write IO)
    chunk_rows = [(0, 1024), (1024, 1536), (1536, 2048)]
    ar_ins = [
        nc.dram_tensor(f"ar_in{c}", [r1 - r0, D], F32).ap()
        for c, (r0, r1) in enumerate(chunk_rows)
    ]
    rs_outs = [
        nc.dram_tensor(f"rs_out{c}", [(r1 - r0) // 2, D], F32).ap()
        for c, (r0, r1) in enumerate(chunk_rows)
    ]

    with tile.TileContext(nc) as tc:
        with (
            tc.tile_pool(name="wgt", bufs=1) as wpool,
            tc.tile_pool(name="qkv", bufs=1) as qkvpool,
            tc.tile_pool(name="pt", bufs=4) as ptpool,
            tc.tile_pool(name="ptd", bufs=2) as ptdpool,
            tc.tile_pool(name="yt", bufs=2) as ytpool,
            tc.tile_pool(name="nrm", bufs=3) as nrmpool,
            tc.tile_pool(name="ob", bufs=3) as obpool,
            tc.tile_pool(name="scp", bufs=2, space="PSUM") as scpool,
            tc.tile_pool(name="avp", bufs=1, space="PSUM") as avpool,
        ):
            # ---- phase 0: load weights/constants ----
            # small constants (ba: gates the Q/K bias adds and thus every
            # score/exp; msk: gates the first masks) go FIRST on the
            # serialized DMA device, then wa/x tiles interleaved so the first
            # QKV matmuls start as early as possible.
            ba_sb = wpool.tile([P, 24], F32, tag="ba")
            nc.sync.dma_start(ba_sb[:], ba[:])
            msk_sb = wpool.tile([P, P], FP8, tag="msk")
            nc.sync.dma_start(msk_sb[:], msk[:])
            wak, xk = [], []
            for t in range(NDT):
                wt_sb = wpool.tile([P, 3 * DL], BF16, tag=f"wak{t}")
                nc.sync.dma_start(wt_sb[:], wa[bass.ts(t, P), :])
                wak.append(wt_sb)
                xt_sb = wpool.tile([P, S], BF16, tag=f"xk{t}")
                nc.sync.dma_start(xt_sb[:, 0 : S // 2], xT[bass.ts(t, P), 0 : S // 2])
                xk.append(xt_sb)
                if t == 0:
                    bv_sb = wpool.tile([P, DL], F32, tag="bv")
                    nc.sync.dma_start(bv_sb[:], bv[:])
            for t in range(NDT):
                nc.sync.dma_start(xk[t][:, S // 2 :], xT[bass.ts(t, P), S // 2 :])
            wpp = []
            for p in range(NP):
                wp_sb = wpool.tile([P, D], BF16, tag=f"wpp{p}")
                nc.sync.dma_start(wp_sb[:], wp[bass.ts(p, P), :])
                wpp.append(wp_sb)
            bp_sb = wpool.tile([P, D], F32, tag="bp")
            nc.sync.dma_start(bp_sb[:], bp[:])
            # fp8 pair tiles for the DR QKV path (first needed ~25us in, so
            # these loads sit behind everything startup-critical)
            xqv, waqv = [], []
            for u in range(NDT // 2):
                xq_sb = wpool.tile([P, 2 * S], FP8, tag=f"xq{u}")
                nc.sync.dma_start(xq_sb[:], xq[bass.ts(u, P), :])
                xqv.append(xq_sb[:].rearrange("p (t n) -> p t n", t=2))
                waq_sb = wpool.tile([P, 2 * 2 * DL], FP8, tag=f"waq{u}")
                nc.sync.dma_start(waq_sb[:], waq[bass.ts(u, P), :])
                waqv.append(waq_sb[:].rearrange("p (t n) -> p t n", t=2))
            mskb_sb = wpool.tile([P, P], BF16, tag="mskb")
            nc.vector.tensor_copy(mskb_sb[:], msk_sb[:])
            ones2_sb = wpool.tile([P, P], FP8, tag="ones2")
            nc.vector.memset(ones2_sb[:], 1.0)
            ones2v = ones2_sb[:].rearrange("p (t m) -> p t m", t=2)
            onesb_sb = wpool.tile([P, HD], BF16, tag="onesb")
            nc.vector.memset(onesb_sb[:], 1.0)
            # pre-zero the pad region [0:128] of every diag-pt pool buffer;
            # steady state keeps it zero (only read afterwards)
            for _b in range(2):
                t1 = ptdpool.tile([P, 1024], FP8, tag="ptd1")
                nc.vector.memset(t1[:, 0:P], 0.0)
                t2 = ptdpool.tile([P, 512], FP8, tag="ptd2")
                nc.vector.memset(t2[:, 0:P], 0.0)
            # prewarm ScalarE's exp table set during the QKV phase so the
            # first attention exp doesn't pay the ACT_TABLE_LOAD
            warm_sb = wpool.tile([1, 2], F32, tag="warm")
            nc.vector.memset(warm_sb[:], 0.0)
            nc.scalar.activation(warm_sb[:, 1:2], warm_sb[:, 0:1],
                                 mybir.ActivationFunctionType.Exp)

            # ---- phase 1: Q^T, K^T  (col-tile m: 0-2 = Q pairs, 3-5 = K pairs)
            # quarter n=0 lives in bf16 [128, 512] tiles (head A on partitions
            # 0:64, B on 64:128); quarters n>=1 come from fp8 DoubleRow
            # matmuls whose M<=64 ISA limit forces [64, head, 1536] tiles
            qt0_t, kt0_t = [None] * NP, [None] * NP
            kt0B_t = [None] * NP  # base-0 copy of head B's K n=0 quarter
            qtn_t, ktn_t = [None] * NP, [None] * NP

            def emit_qk(m, n):
                # one quarter (512 cols) of one QKV^T col-tile — a ~1us PE
                # unit so it can slot into attention pipeline gaps
                ps = scpool.tile([P, 1024], F32, tag="sc")
                if n == 0:
                    dst = qkvpool.tile([P, QT], BF16, tag=f"qkv0T{m}", name=f"qkv0T{m}")
                    if m < NP:
                        qt0_t[m] = dst
                    else:
                        kt0_t[m - NP] = dst
                    for t in range(NDT):
                        nc.tensor.matmul(
                            ps[:, 0:QT],
                            lhsT=wak[t][:, bass.ts(m, P)],
                            rhs=xk[t][:, bass.ts(n, QT)],
                            start=(t == 0),
                            stop=(t == NDT - 1),
                        )
                    nc.vector.tensor_scalar_add(
                        dst[:], ps[:, 0:QT], ba_sb[:, m : m + 1]
                    )
                    if m >= NP:
                        # head-B rows live at partitions 64:128, but the fp8-DR
                        # q tiles are base-0: scores mixing them need a base-0
                        # copy of this quarter (matmul operands must share base)
                        kb = qkvpool.tile([HD, QT], BF16, tag=f"kt0B{m}", name=f"kt0B{m}")
                        kt0B_t[m - NP] = kb
                        nc.sync.dma_start(kb[:], dst[HD:P, :])
                    return
                if n == 1:
                    dst = qkvpool.tile([HD, 2 * 3 * QT], BF16, tag=f"qkvnT{m}", name=f"qkvnT{m}")
                    if m < NP:
                        qtn_t[m] = dst
                    else:
                        ktn_t[m - NP] = dst
                else:
                    dst = qtn_t[m] if m < NP else ktn_t[m - NP]
                dv = dst[:].rearrange("p (h n) -> p h n", h=2)
                for h2 in range(2):
                    for u in range(NDT // 2):
                        nc.tensor.matmul(
                            ps[0:HD, bass.ts(h2, QT)],
                            lhsT=waqv[u][:, :, m * P + h2 * HD : m * P + (h2 + 1) * HD],
                            rhs=xqv[u][:, :, bass.ts(n, QT)],
                            start=(u == 0),
                            stop=(u == NDT // 2 - 1),
                            perf_mode=DR,
                        )
                for h2 in range(2):
                    nc.vector.tensor_scalar_add(
                        dv[:, h2, bass.ts(n - 1, QT)],
                        ps[0:HD, bass.ts(h2, QT)],
                        ba_sb[0:HD, 8 + 2 * m + h2 : 9 + 2 * m + h2],
                    )

            # V j-pair tiles (fp8): v_t[u] holds s-tiles (2u+1, 2u) in free
            # slots (t=0 -> odd, t=1 -> even) to match the score-pair order.
            # u 0/1 also get bf16 copies: q-tile i=0 (few attended keys, so
            # quantization noise doesn't average out) runs a bf16 AV path.
            v_t = [None] * (S // P // 2)
            v8_t = [None, None]

            def emit_v(s):
                ps = scpool.tile([P, 1024], F32, tag="sc")
                for t in range(NDT):
                    nc.tensor.matmul(
                        ps[:, 0:DL],
                        lhsT=xk[t][:, bass.ts(s, P)],
                        rhs=wak[t][:, 2 * DL : 3 * DL],
                        start=(t == 0),
                        stop=(t == NDT - 1),
                    )
                u, odd = divmod(s, 2)
                if v_t[u] is None:
                    v_t[u] = qkvpool.tile([P, HL * 2 * HD], FP8, tag=f"v{u}", name=f"v{u}")
                vt4 = v_t[u][:].rearrange("p (h t x) -> p h t x", h=HL, t=2)
                nc.vector.tensor_add(
                    vt4[:, :, 0 if odd else 1, :],
                    ps[:, 0:DL].rearrange("p (h x) -> p h x", h=HL),
                    bv_sb[:].rearrange("p (h x) -> p h x", h=HL),
                )
                if u < 2:
                    if v8_t[u] is None:
                        v8_t[u] = qkvpool.tile([P, HL * 2 * HD], BF16, tag=f"v8{u}", name=f"v8{u}")
                    v8t4 = v8_t[u][:].rearrange("p (h t x) -> p h t x", h=HL, t=2)
                    nc.vector.tensor_add(
                        v8t4[:, :, 0 if odd else 1, :],
                        ps[:, 0:DL].rearrange("p (h x) -> p h x", h=HL),
                        bv_sb[:].rearrange("p (h x) -> p h x", h=HL),
                    )

            yt_t = [None] * NP

            def emit_attn(i, p, fillers=()):
                hA, hB = 2 * p, 2 * p + 1
                avA = avpool.tile([HD, 1024], F32, tag="avA")
                avB = avpool.tile([HD, 1024], F32, tag="avB")
                n_pairs = 2 * i + 2
                scs = {}

                # score blocks per pair u: (j, col_off, q0, n); block order is
                # (odd j, even j) to match the V-pair layout and to make the
                # diagonal pairs' valid region contiguous
                def blocks_of(u):
                    if u < 2 * i:
                        return [(2 * u + 1, 0, i * QT, QT), (2 * u, QT, i * QT, QT)]
                    if u == 2 * i:
                        return [(4 * i + 1, P, i * QT + P, 384), (4 * i, QT, i * QT, QT)]
                    return [(4 * i + 3, P, i * QT + 384, P), (4 * i + 2, 256, i * QT + 256, 256)]

                def kt_ap(hsel, j):
                    if j < 4:
                        if hsel == 0:
                            return kt0_t[p][0:HD, bass.ts(j, P)]
                        if i == 0:
                            return kt0_t[p][HD:P, bass.ts(j, P)]
                        return kt0B_t[p][:, bass.ts(j, P)]
                    kv = ktn_t[p][:].rearrange("q (h n) -> q h n", h=2)
                    return kv[:, hsel, (j - 4) * P : (j - 3) * P]

                def qt_ap(hsel, q0, n):
                    if q0 + n <= QT:
                        return qt0_t[p][hsel * HD : (hsel + 1) * HD, q0 : q0 + n]
                    qv = qtn_t[p][:].rearrange("q (h n) -> q h n", h=2)
                    return qv[:, hsel, q0 - QT : q0 - QT + n]

                def emit_scores(u):
                    scA = scpool.tile([P, 1024], F32, tag="sc")
                    scB = scpool.tile([P, 1024], F32, tag="sc")
                    for j, off, q0, n in blocks_of(u):
                        for sc, hsel in ((scA, 0), (scB, 1)):
                            nc.tensor.matmul(
                                sc[:, off : off + n],
                                lhsT=kt_ap(hsel, j),
                                rhs=qt_ap(hsel, q0, n),
                                start=True, stop=True,
                            )
                    scs[u] = (scA, scB)

                def process(u):
                    scA, scB = scs.pop(u)
                    bf = i == 0  # bf16 AV path for the first q-tile
                    PDT = BF16 if bf else FP8
                    mtile = mskb_sb if bf else msk_sb
                    if u < 2 * i:
                        e0, e1 = 0, 1024
                        ptA = ptpool.tile([P, 1024], PDT, tag="pt")
                        ptB = ptpool.tile([P, 1024], PDT, tag="pt")
                        masks = []
                        qoff, qn = 0, QT
                    elif u == 2 * i:
                        e0, e1 = P, 1024
                        ptA = ptdpool.tile([P, 1024], PDT, tag="ptd1b" if bf else "ptd1")
                        ptB = ptdpool.tile([P, 1024], PDT, tag="ptd1b" if bf else "ptd1")
                        masks = [(P, 256), (QT, QT + P)]
                        qoff, qn = 0, QT
                    else:
                        e0, e1 = P, 512
                        ptA = ptdpool.tile([P, 512], PDT, tag="ptd2b" if bf else "ptd2")
                        ptB = ptdpool.tile([P, 512], PDT, tag="ptd2b" if bf else "ptd2")
                        masks = [(P, 256), (256, 384)]
                        qoff, qn = 256, 256
                    for sc, pt in ((scA, ptA), (scB, ptB)):
                        nc.scalar.activation(pt[:, e0:e1], sc[:, e0:e1],
                                             mybir.ActivationFunctionType.Exp, scale=0.125)
                        for m0, m1 in masks:
                            nc.vector.tensor_mul(pt[:, m0:m1], pt[:, m0:m1], mtile[:])
                    first, last = (u == 0), (u == n_pairs - 1)
                    if bf:
                        # plain bf16 matmuls over the exact valid block ranges
                        # (block order: j0-full first so `start` covers the
                        # whole region)
                        v8t4 = v8_t[u][:].rearrange("p (h t x) -> p h t x", h=HL, t=2)
                        if u == 0:
                            blks = [(1, QT, 0, QT), (0, P, P, 384)]
                        else:
                            blks = [(1, 256, 256, 256), (0, P, 384, P)]
                        for pt, av, h in ((ptA, avA, hA), (ptB, avB, hB)):
                            for bi, (t, off, ql, n) in enumerate(blks):
                                st = first and bi == 0
                                sp = last and bi == len(blks) - 1
                                nc.tensor.matmul(
                                    av[:, ql : ql + n],
                                    lhsT=v8t4[:, h, t, :], rhs=pt[:, off : off + n],
                                    start=st, stop=sp,
                                )
                                nc.tensor.matmul(
                                    av[:, QT + ql : QT + ql + n],
                                    lhsT=onesb_sb[:], rhs=pt[:, off : off + n],
                                    start=st, stop=sp,
                                )
                    else:
                        for pt, av, h in ((ptA, avA, hA), (ptB, avB, hB)):
                            rhs = pt[:, 0 : 2 * qn].rearrange("p (t n) -> p t n", t=2)
                            vt4 = v_t[u][:].rearrange("p (h t x) -> p h t x", h=HL, t=2)
                            nc.tensor.matmul(
                                av[:, qoff : qoff + qn], lhsT=vt4[:, h, :, :], rhs=rhs,
                                start=first, stop=last, perf_mode=DR,
                            )
                            nc.tensor.matmul(
                                av[:, QT + qoff : QT + qoff + qn], lhsT=ones2v, rhs=rhs,
                                start=first, stop=last, perf_mode=DR,
                            )

                emit_scores(0)
                for u in range(n_pairs):
                    if u + 1 < n_pairs:
                        emit_scores(u + 1)
                    process(u)
                    # PE filler work (QKV/V/proj/RS units) slotted into the
                    # Act-paced attention pipeline
                    if u < len(fillers):
                        for th in fillers[u]:
                            th()

                # normalize: n is partition-replicated in av[:, 512:1024], so
                # ytp = y_u * recip(n) elementwise; head B bounced via DMA to
                # reach partitions 64:128
                ytp = ytpool.tile([P, QT], BF16, tag=f"yt{p}")
                rbA = nrmpool.tile([HD, QT], F32, tag="rbA")
                nc.vector.reciprocal(rbA[:], avA[:, QT:])
                nc.vector.tensor_mul(ytp[0:HD, :], avA[:, 0:QT], rbA[:])
                rbB = nrmpool.tile([HD, QT], F32, tag="rbB")
                nc.vector.reciprocal(rbB[:], avB[:, QT:])
                tmpB = nrmpool.tile([HD, QT], BF16, tag="tmpB")
                nc.vector.tensor_mul(tmpB[:], avB[:, 0:QT], rbB[:])
                nc.sync.dma_start(ytp[HD:P, :], tmpB[:])
                yt_t[p] = ytp

            # proj consumes the i-th q-tile's yt (stashed per i so filler-
            # scheduled proj units read the right generation)
            yt_gen = [None] * NI

            def emit_proj_ss(i, ss):
                yts = yt_gen[i]
                ps = scpool.tile([P, 1024], F32, tag="sc")
                for p in range(NP):
                    nc.tensor.matmul(
                        ps[:, 0:512],
                        lhsT=yts[p][:, bass.ts(ss, P)],
                        rhs=wpp[p][:, 0:512],
                        start=(p == 0), stop=(p == NP - 1),
                    )
                    nc.tensor.matmul(
                        ps[:, 512:768],
                        lhsT=yts[p][:, bass.ts(ss, P)],
                        rhs=wpp[p][:, 512:768],
                        start=(p == 0), stop=(p == NP - 1),
                    )
                ob = obpool.tile([P, D], F32, tag="ob")
                nc.vector.tensor_add(ob[:], ps[:, 0:D], bp_sb[:])
                row0 = i * QT + ss * P
                for c, (r0, r1) in enumerate(chunk_rows):
                    if r0 <= row0 < r1:
                        nc.sync.dma_start(ar_ins[c][row0 - r0 : row0 - r0 + P, :], ob[:])

            def emit_rs(c):
                # ReduceScatter(add) over the core pair: even core receives the
                # summed first half of the chunk, odd core the second half;
                # result bounced through SBUF into `out` (host stitches)
                nc.gpsimd.collective_compute(
                    "ReduceScatter",
                    mybir.AluOpType.add,
                    replica_groups=[[0, 1], [2, 3], [4, 5], [6, 7]],
                    ins=[ar_ins[c][:, :].opt()],
                    outs=[rs_outs[c][:, :].opt()],
                )
                base = [0, 512, 768][c]
                nrows = (chunk_rows[c][1] - chunk_rows[c][0]) // 2
                for r in range(0, nrows, P):
                    oc = obpool.tile([P, D], F32, tag="oc")
                    nc.sync.dma_start(oc[:], rs_outs[c][r : r + P, :])
                    nc.sync.dma_start(out[base + r : base + r + P, :], oc[:])

            # ---- main interleaved schedule ----
            # attention for q-tile i only needs Q/K col-quarters n <= i and V
            # s-tiles < 4(i+1), so QKV/V/proj/RS units slot into the Act-paced
            # attention pipeline as per-group PE fillers (engine execution
            # follows emission order, so placement here IS the schedule)
            def Q(m, n):
                return lambda: emit_qk(m, n)

            def V(s):
                return lambda: emit_v(s)

            def PJ(i, ss):
                return lambda: emit_proj_ss(i, ss)

            def RS(c):
                return lambda: emit_rs(c)

            for _rep in range(REPEAT):
                emit_qk(0, 0)
                emit_qk(3, 0)
                for s in range(4):
                    emit_v(s)
                emit_attn(0, 0, [[Q(1, 0)], [Q(4, 0)]])
                emit_attn(0, 1, [[Q(2, 0)], [Q(5, 0)]])
                emit_attn(0, 2, [[Q(0, 1)], [Q(3, 1)]])
                yt_gen[0] = list(yt_t)
                emit_attn(1, 0, [[V(4), V(5)], [V(6), V(7)], [Q(1, 1)], [Q(4, 1)]])
                emit_attn(1, 1, [[Q(2, 1)], [Q(5, 1)], [PJ(0, 0)], [PJ(0, 1)]])
                emit_attn(1, 2, [[Q(0, 2)], [Q(3, 2)], [PJ(0, 2)], [PJ(0, 3)]])
                yt_gen[1] = list(yt_t)
                emit_attn(2, 0, [[V(8), V(9)], [V(10), V(11)], [Q(1, 2)], [Q(4, 2)], [PJ(1, 0)], [PJ(1, 1)]])
                emit_attn(2, 1, [[Q(2, 2)], [Q(5, 2)], [PJ(1, 2)], [PJ(1, 3)], [RS(0)], [Q(0, 3)]])
                emit_attn(2, 2, [[Q(3, 3)], [Q(1, 3)], [Q(4, 3)], [Q(2, 3)], [Q(5, 3)], []])
                yt_gen[2] = list(yt_t)
                emit_attn(3, 0, [[V(12), V(13)], [V(14), V(15)], [PJ(2, 0)], [PJ(2, 1)], [PJ(2, 2)], [PJ(2, 3)], [RS(1)], []])
                emit_attn(3, 1, [])
                emit_attn(3, 2, [])
                yt_gen[3] = list(yt_t)
                for ss in range(4):
                    emit_proj_ss(3, ss)
                emit_rs(2)

    _legalize_waits(nc)
    return nc


_NC_CACHE = {}


def _get_nc():
    if "nc" not in _NC_CACHE:
        _NC_CACHE["nc"] = _build()
    return _NC_CACHE["nc"]


def _prep_inputs(x, W_attn, b_attn, W_proj, b_proj):
    bf = ml_dtypes.bfloat16
    fp8 = ml_dtypes.float8_e4m3
    x = np.asarray(x, np.float32)
    W_attn = np.asarray(W_attn, np.float32)
    b_attn = np.asarray(b_attn, np.float32)
    W_proj = np.asarray(W_proj, np.float32)
    b_proj = np.asarray(b_proj, np.float32)

    # stair mask for the 128-col diagonal of each diag j-block
    mask = (np.arange(P)[None, :] >= np.arange(P)[:, None]).astype(fp8)

    in_maps = []
    for c in range(N_CORES):
        b, g = divmod(c, 2)
        cols = slice(DL * g, DL * g + DL)
        xT = np.ascontiguousarray(x[b].T).astype(bf)
        wa = np.concatenate(
            [W_attn[:, 0:D][:, cols], W_attn[:, D : 2 * D][:, cols], W_attn[:, 2 * D :][:, cols]],
            axis=1,
        ).astype(bf)
        ba_sl = np.concatenate(
            [b_attn[0:D][cols], b_attn[D : 2 * D][cols], b_attn[2 * D :][cols]]
        ).astype(np.float32)
        ba2 = np.ascontiguousarray(ba_sl[: 2 * DL].reshape(6, P).T)
        ba9 = np.zeros((P, 24), np.float32)
        ba9[:, :6] = ba2
        # cols 8..19: per-(m, head-half) bias for the [64, head, n] DR layout
        ba9[:HD, 8:20] = ba_sl[: 2 * DL].reshape(12, HD).T
        # fp8 pair-interleaved x^T and Q/K weight columns for DoubleRow
        fp8c = ml_dtypes.float8_e4m3
        x_pairs = (
            np.ascontiguousarray(x[b].T.reshape(3, 2, P, S).transpose(0, 2, 1, 3))
            .reshape(D // 2, 2 * S)
            .astype(fp8c)
        )
        wqk = np.concatenate([W_attn[:, 0:D][:, cols], W_attn[:, D : 2 * D][:, cols]], axis=1)
        wa_pairs = (
            np.ascontiguousarray(wqk.reshape(3, 2, P, 2 * DL).transpose(0, 2, 1, 3))
            .reshape(D // 2, 2 * 2 * DL)
            .astype(fp8c)
        )
        bv_b = np.ascontiguousarray(np.broadcast_to(ba_sl[2 * DL :], (P, DL))).astype(np.float32)
        wp_c = np.ascontiguousarray(W_proj[cols, :]).astype(bf)
        bp_full = b_proj if g == 0 else np.zeros_like(b_proj)
        bp_b = np.ascontiguousarray(np.broadcast_to(bp_full, (P, D))).astype(np.float32)
        in_maps.append(
            {
                "xT": xT,
                "wa": wa,
                "xq": x_pairs,
                "waq": wa_pairs,
                "ba": ba9,
                "bv": bv_b,
                "wp": wp_c,
                "bp": bp_b,
                "msk": mask,
            }
        )
    return in_maps


def kernel(x, W_attn, b_attn, W_proj, b_proj):
    in_maps = _prep_inputs(x, W_attn, b_attn, W_proj, b_proj)
    nc = _get_nc()
    res = run_bass_kernel_spmd(nc, in_maps, list(range(N_CORES)))
    # stitch: chunk c (global rows r0:r1) -> even core's out rows
    # [base:base+h] = summed r0:r0+h, odd core's = r0+h:r1
    outs = []
    for b in range(B):
        rows = []
        for c, (r0, r1) in enumerate([(0, 1024), (1024, 1536), (1536, 2048)]):
            h = (r1 - r0) // 2
            base = [0, 512, 768][c]
            rows.append(res.results[2 * b]["out"][base : base + h])
            rows.append(res.results[2 * b + 1]["out"][base : base + h])
        outs.append(np.concatenate(rows, axis=0))
    return np.stack(outs).astype(np.float32)
